# revision 5
# baseline (speedup 1.0000x reference)
"""GatedLinearRecurrence Trainium2 kernel (8-core SPMD, Bass/Tile).

Strategy: shard (batch=2) x (4 sequence chunks of 1024 tokens) across 8 cores.
Each core processes 1152 tokens: a 128-token warm-up window (re-computed
redundantly; the recurrence decay makes carry-in truncation error ~1e-24)
followed by its 1024 "main" tokens.  No collectives needed.

Per-core pipeline (channels-on-partitions, tokens-on-free layout):
  LN(x) [t,d] -> PE-transpose (bf16) -> x̂T [d,t] -> in_proj (bf16 matmul)
  -> causal depthwise conv (bf16, 4 shifted tensor_scalar ops) -> silu -> mask
  -> fp8 quantized copy -> gate matmul (fp8e4m3 DoubleRow, 2 k-tiles/instr)
  -> sigmoid (dequant via activation scale) -> b=-(1-a)*xc
  -> tensor_tensor_scan (h=-h, fp32 carry) -> y*silu(z) -> out_proj (bf16)
  -> residual subtract -> out [t,dm].

Dtype plan (validated numerically: rel err ~5e-3 vs 2e-2 budget):
  - gate matmul fp8e4m3 both operands (weights pre-scaled x512 on host,
    activations x16 in-kernel; sigmoid activation scale 1/8192 dequantizes).
    DoubleRow perf mode halves PE time for this largest matmul.
  - everything else bf16 (same PE rate as f32r, half the DMA + DVE cost).
  - silu(z) stays resident in SBUF (no HBM scratch roundtrip).

The sign trick: scan data1 = (a-1)*x_conv = -b gives -h; -h*silu(z) = -yg;
out = x - matmul(-yg) = x + proj(yg).
"""
import sys

for p in ("/opt/trn_rl_repo", "/root/.axon_site/_ro/trn_rl_repo"):
    if p not in sys.path:
        sys.path.insert(0, p)

import numpy as np
import ml_dtypes

import concourse.bass as bass
import concourse.bacc as bacc
import concourse.tile as tile
import concourse.mybir as mybir
from concourse.bass_utils import run_bass_kernel_spmd
from concourse.masks import make_identity

F32 = mybir.dt.float32
BF16 = mybir.dt.bfloat16
F8 = mybir.dt.float8e4
AF = mybir.ActivationFunctionType
OP = mybir.AluOpType
DR = mybir.MatmulPerfMode.DoubleRow

B, L, D = 2, 4096, 1024
DI = 2048            # d_inner
NT = 1152            # tokens per core (128 warm-up + 1024 main)
W = 128              # warm-up tokens
CHUNK = 1024
NTT = NT // 128      # 9 token tiles
KD = D // 128        # 8 k-tiles over d_model
KC = DI // 128       # 16 k-tiles over d_inner
TC = 384             # matmul N chunk (3 per core)
NTC = NT // TC
EPS = 1e-5
S_GW = 512.0         # host-side fp8 scale on gate weights
S_XC = 16.0          # in-kernel fp8 scale on x_conv
DEQ = 1.0 / (S_GW * S_XC)
NB = 512             # out_proj column half width

_cache = {}


def _build():
    nc = bacc.Bacc(None, target_bir_lowering=False)

    x_h = nc.dram_tensor("x", [NT, D], F32, kind="ExternalInput")
    w1x_h = nc.dram_tensor("w1x", [D, DI], BF16, kind="ExternalInput")
    w1z_h = nc.dram_tensor("w1z", [D, DI], BF16, kind="ExternalInput")
    gw_h = nc.dram_tensor("gw", [DI, DI], F8, kind="ExternalInput")
    op_h = nc.dram_tensor("opw", [DI, D], BF16, kind="ExternalInput")
    convw_h = nc.dram_tensor("convw", [128, KC * 4], F32, kind="ExternalInput")
    convb_h = nc.dram_tensor("convb", [128, KC], F32, kind="ExternalInput")
    gateb_h = nc.dram_tensor("gateb", [128, KC], F32, kind="ExternalInput")
    normb_h = nc.dram_tensor("normb", [128, KD], F32, kind="ExternalInput")
    mask_h = nc.dram_tensor("mask", [1, NT], BF16, kind="ExternalInput")
    out_h = nc.dram_tensor("out", [CHUNK, D], F32, kind="ExternalOutput")

    with tile.TileContext(nc) as tc:
        with tc.tile_pool(name="consts", bufs=1) as consts:

            ident = consts.tile([128, 128], BF16, name="ident")
            make_identity(nc, ident)
            mask_sb = consts.tile([128, W], BF16, name="mask_sb")
            nc.gpsimd.dma_start(
                out=mask_sb,
                in_=bass.AP(tensor=mask_h, offset=0, ap=[[0, 128], [1, W]]),
            )
            convw = consts.tile([128, KC * 4], F32, name="convw")
            nc.gpsimd.dma_start(out=convw, in_=convw_h.ap())
            convb = consts.tile([128, KC], F32, name="convb")
            nc.gpsimd.dma_start(out=convb, in_=convb_h.ap())
            gateb = consts.tile([128, KC], F32, name="gateb")
            nc.gpsimd.dma_start(out=gateb, in_=gateb_h.ap())
            normb = consts.tile([128, KD], F32, name="normb")
            nc.gpsimd.dma_start(out=normb, in_=normb_h.ap())
            eps_t = consts.tile([128, 1], F32, name="eps_t")
            nc.vector.memset(eps_t, EPS)

            with tc.tile_pool(name="xcp", bufs=1) as xcp, \
                 tc.tile_pool(name="zsp", bufs=1) as zsp:
                xc = [xcp.tile([128, NT], BF16, name=f"xct{e}") for e in range(KC)]
                # fp8 pair tiles for the DoubleRow gate matmul: pair j holds
                # channel tiles 2j (dim1=0) and 2j+1 (dim1=1), scaled by S_XC
                xc8 = [xcp.tile([128, 2, NT], F8, name=f"xc8_{j}")
                       for j in range(KC // 2)]
                zs = [zsp.tile([128, CHUNK], BF16, name=f"zs{e}") for e in range(KC)]

                # ---- S1-S3: LN, transpose, in_proj (x & z), conv, silu ----
                with tc.tile_pool(name="xT", bufs=1) as xTp, \
                     tc.tile_pool(name="s1roll", bufs=2) as s1r, \
                     tc.tile_pool(name="stat", bufs=4) as stp, \
                     tc.tile_pool(name="w1s", bufs=3) as ws, \
                     tc.tile_pool(name="psmm", bufs=4, space="PSUM") as psmm, \
                     tc.tile_pool(name="psz", bufs=2, space="PSUM") as pszp, \
                     tc.tile_pool(name="pstr", bufs=2, space="PSUM") as pstr:

                    # x-hat-T chunk tiles [d-tile][t-chunk]: finer deps, so
                    # the first in_proj matmuls start after 3 LN iterations.
                    xT = [[xTp.tile([128, TC], BF16, name=f"xTt{d_}_{c_}")
                           for c_ in range(NTC)] for d_ in range(KD)]

                    # prefetch the first in_proj weight tiles on the (idle)
                    # scalar queue so the PE isn't stuck behind the 4.6MB of
                    # x token-tile loads on the sync queue
                    wts, xins = {}, {}
                    for et in range(4):
                        wt = ws.tile([128, KD, 128], BF16, tag="w1",
                                     bufs=8, name=f"wt{et}")
                        nc.scalar.dma_start(
                            out=wt,
                            in_=w1x_h.ap()[:, et * 128:(et + 1) * 128]
                            .rearrange("(kt p) e -> p kt e", p=128))
                        wts[et] = wt

                    for it in range(NTT):
                        tc3, col = it // 3, (it % 3) * 128
                        xt = s1r.tile([128, D], F32, tag="xt", bufs=3, name="xt")
                        nc.sync.dma_start(out=xt, in_=x_h.ap()[it * 128:(it + 1) * 128, :])
                        stats = stp.tile([128, 2, 6], F32, tag="stats", name="stats")
                        nc.vector.bn_stats(out=stats[:, 0, :], in_=xt[:, 0:512])
                        nc.vector.bn_stats(out=stats[:, 1, :], in_=xt[:, 512:1024])
                        mv = stp.tile([128, 2], F32, tag="mv", name="mv")
                        nc.vector.bn_aggr(out=mv, in_=stats)
                        rstd = stp.tile([128, 1], F32, tag="rstd", name="rstd")
                        nc.scalar.activation(out=rstd, in_=mv[:, 1:2], func=AF.Sqrt,
                                             bias=eps_t, scale=1.0)
                        nc.vector.reciprocal(out=rstd, in_=rstd)
                        xtn = s1r.tile([128, D], BF16, tag="xtn", bufs=3, name="xtn")
                        nc.vector.tensor_scalar(out=xtn, in0=xt, scalar1=mv[:, 0:1],
                                                scalar2=rstd, op0=OP.subtract, op1=OP.mult)
                        for d_ in range(KD):
                            pst = pstr.tile([128, 128], BF16, tag="tr", name="pst")
                            nc.tensor.transpose(pst, xtn[:, d_ * 128:(d_ + 1) * 128], ident)
                            # evac + norm_b on the Scalar engine
                            nc.scalar.activation(
                                out=xT[d_][tc3][:, col:col + 128], in_=pst,
                                func=AF.Identity, bias=normb[:, d_:d_ + 1], scale=1.0)

                    # in_proj x-half + conv + silu + warm-up mask + fp8 copy.
                    # The first 4 ets interleave their t-chunks so the PE never
                    # waits on layernorm chunks still in flight.
                    NW = 4
                    order = [(e, c) for c in range(NTC) for e in range(NW)]
                    order += [(e, c) for e in range(NW, KC) for c in range(NTC)]

                    def s2_chain(et, tc3):
                        if tc3 == 0:
                            if et not in wts:
                                wt = ws.tile([128, KD, 128], BF16, tag="w1",
                                             bufs=8, name=f"wt{et}")
                                nc.sync.dma_start(
                                    out=wt,
                                    in_=w1x_h.ap()[:, et * 128:(et + 1) * 128]
                                    .rearrange("(kt p) e -> p kt e", p=128))
                                wts[et] = wt
                            xin = s1r.tile([128, NT + 3], BF16, tag="xin",
                                           bufs=NW + 2, name=f"xin{et}")
                            nc.vector.memset(xin[:, 0:3], 0.0)
                            xins[et] = xin
                        ps = psmm.tile([128, TC], F32, tag="mm", name="ps")
                        for kt in range(KD):
                            nc.tensor.matmul(
                                ps, wts[et][:, kt, :], xT[kt][tc3],
                                start=(kt == 0), stop=(kt == KD - 1))
                        nc.scalar.copy(
                            out=xins[et][:, 3 + tc3 * TC: 3 + (tc3 + 1) * TC], in_=ps)
                        if tc3 == NTC - 1:
                            wts.pop(et)
                            xin = xins.pop(et)
                            tmp = s1r.tile([128, NT], BF16, tag="ctmp", name="ctmp")
                            nc.vector.tensor_scalar_mul(
                                tmp, xin[:, 0:NT], convw[:, et * 4:et * 4 + 1])
                            for k in range(1, 4):
                                nc.vector.scalar_tensor_tensor(
                                    out=tmp, in0=xin[:, k:k + NT],
                                    scalar=convw[:, et * 4 + k:et * 4 + k + 1],
                                    in1=tmp, op0=OP.mult, op1=OP.add)
                            nc.scalar.activation(out=xc[et], in_=tmp, func=AF.Silu,
                                                 bias=convb[:, et:et + 1], scale=1.0)
                            # mask is non-unit only on the warm-up columns
                            nc.vector.tensor_mul(
                                xc[et][:, 0:W], xc[et][:, 0:W], mask_sb)
                            # quantized copy for the DoubleRow gate matmul
                            nc.gpsimd.tensor_scalar_mul(
                                xc8[et // 2][:, et % 2, :], xc[et], S_XC)

                    for et, tc3 in order:
                        s2_chain(et, tc3)

                    # in_proj z-half + silu -> SBUF-resident zs (skip warm-up:
                    # the first chunk starts at token W)
                    for et in range(KC):
                        wt = ws.tile([128, KD, 128], BF16, tag="w1", bufs=8, name="wtz")
                        nc.sync.dma_start(
                            out=wt,
                            in_=w1z_h.ap()[:, et * 128:(et + 1) * 128]
                            .rearrange("(kt p) e -> p kt e", p=128))
                        for tc3 in range(NTC):
                            lo = max(tc3 * TC, W)
                            n = (tc3 + 1) * TC - lo
                            ps = pszp.tile([128, TC], F32, tag="mmz", name="psz")
                            for kt in range(KD):
                                nc.tensor.matmul(
                                    ps[:, 0:n], wt[:, kt, :], xT[kt][tc3][:, lo - tc3 * TC:TC],
                                    start=(kt == 0), stop=(kt == KD - 1))
                            nc.scalar.activation(
                                out=zs[et][:, lo - W:(tc3 + 1) * TC - W],
                                in_=ps[:, 0:n], func=AF.Silu)

                # ---- S4-S6: gate matmul (fp8 DoubleRow), sigmoid, chunked
                # scan, y*silu(z).  y is kept in SBUF as chunk tiles whose
                # column slices feed out_proj directly as stationary operands.
                with tc.tile_pool(name="yp", bufs=1) as yp, \
                     tc.tile_pool(name="opwp", bufs=1) as opp:
                    ych = [[None] * NTC for _ in range(KC)]
                    # full out_proj weight prefetch (bf16, 4.2MB) on the
                    # scalar queue while the PE runs the gate matmuls
                    opw_sb = opp.tile([128, KC, D], BF16, name="opw_sb")
                    nc.scalar.dma_start(
                        out=opw_sb,
                        in_=op_h.ap().rearrange("(kt p) d -> p kt d", p=128))

                    with tc.tile_pool(name="gws", bufs=3) as gs, \
                         tc.tile_pool(name="ach", bufs=6) as ayp, \
                         tc.tile_pool(name="s6roll", bufs=6) as s6r, \
                         tc.tile_pool(name="psg", bufs=4, space="PSUM") as psg:

                        for et in range(KC):
                            gt = gs.tile([128, KC, 128], F8, tag="gw", name="gt")
                            nc.sync.dma_start(
                                out=gt,
                                in_=gw_h.ap()[:, et * 128:(et + 1) * 128]
                                .rearrange("(kt p) e -> p kt e", p=128))
                            prev_y = None
                            for tc3 in range(NTC):
                                a_t = ayp.tile([128, TC], BF16, tag="ach", name="ach")
                                ps = psg.tile([128, TC], F32, tag="mm", name="psgt")
                                for j in range(KC // 2):
                                    nc.tensor.matmul(
                                        ps, gt[:, 2 * j:2 * j + 2, :],
                                        xc8[j][:, :, tc3 * TC:(tc3 + 1) * TC],
                                        start=(j == 0), stop=(j == KC // 2 - 1),
                                        perf_mode=DR)
                                nc.scalar.activation(
                                    out=a_t, in_=ps,
                                    func=AF.Sigmoid, bias=gateb[:, et:et + 1],
                                    scale=DEQ)
                                bt = s6r.tile([128, TC], BF16, tag="bt", name="bt")
                                nc.vector.scalar_tensor_tensor(
                                    out=bt, in0=a_t, scalar=1.0,
                                    in1=xc[et][:, tc3 * TC:(tc3 + 1) * TC],
                                    op0=OP.subtract, op1=OP.mult)
                                y_t = yp.tile([128, TC], BF16, name=f"y{et}_{tc3}")
                                init = 0.0 if tc3 == 0 else prev_y[:, TC - 1:TC]
                                nc.vector.tensor_tensor_scan(
                                    out=y_t, data0=a_t, data1=bt, initial=init,
                                    op0=OP.mult, op1=OP.add)
                                ych[et][tc3] = y_t
                                prev_y = y_t
                            # -yg: multiply after the carry chain is complete
                            for tc3 in range(NTC):
                                lo = max(tc3 * TC, W)
                                nc.vector.tensor_mul(
                                    ych[et][tc3][:, lo - tc3 * TC:TC],
                                    ych[et][tc3][:, lo - tc3 * TC:TC],
                                    zs[et][:, lo - W:(tc3 + 1) * TC - W])

                    # ---- S7: out_proj + residual.  y chunk slices are the
                    # stationary operands; 4 passes of (column half x 4 token
                    # blocks), 4 rolling PSUM banks so pass n+1 matmuls
                    # overlap pass n evac. ----
                    def yslice(kt, tb):
                        col = W + tb * 128          # absolute column in [0, NT)
                        tc3, off = col // TC, col % TC
                        return ych[kt][tc3][:, off:off + 128]

                    with tc.tile_pool(name="s7roll", bufs=6) as s7r, \
                         tc.tile_pool(name="s7res", bufs=8) as s7x, \
                         tc.tile_pool(name="psop", bufs=4, space="PSUM") as psop:
                        for nb in range(2):
                            for tbg in range(2):
                                tbs = [tbg * 4 + i for i in range(4)]
                                xres = {}
                                for tb in tbs:
                                    xres[tb] = s7x.tile([128, NB], F32, tag="xres",
                                                        name=f"xres{nb}_{tb}")
                                    nc.scalar.dma_start(
                                        out=xres[tb],
                                        in_=x_h.ap()[W + tb * 128:W + (tb + 1) * 128,
                                                     nb * NB:(nb + 1) * NB])
                                pss = {tb: psop.tile([128, NB], F32, tag="op",
                                                     name=f"pso{tb}") for tb in tbs}
                                for kt in range(KC):
                                    for tb in tbs:
                                        nc.tensor.matmul(
                                            pss[tb], yslice(kt, tb),
                                            opw_sb[:, kt, nb * NB:(nb + 1) * NB],
                                            start=(kt == 0), stop=(kt == KC - 1))
                                for tb in tbs:
                                    oh = s7r.tile([128, NB], F32, tag="oh", name="oh")
                                    nc.vector.tensor_sub(oh, xres[tb], pss[tb])
                                    nc.sync.dma_start(
                                        out=out_h.ap()[tb * 128:(tb + 1) * 128,
                                                       nb * NB:(nb + 1) * NB],
                                        in_=oh)

    nc.compile()
    return nc


def _prep_host(x, norm_w, norm_b, in_proj_w, conv_w, conv_b, gate_w, gate_b,
               out_proj_w):
    w1 = (in_proj_w * norm_w[None, :]).astype(np.float32)
    w1xT = np.ascontiguousarray(w1[:DI].T).astype(ml_dtypes.bfloat16)   # [D, DI]
    w1zT = np.ascontiguousarray(w1[DI:].T).astype(ml_dtypes.bfloat16)   # [D, DI]
    gwT = np.ascontiguousarray(gate_w.T)             # [DI, DI]
    gw8 = np.clip(gwT * S_GW, -240, 240).astype(ml_dtypes.float8_e4m3)
    opT = np.ascontiguousarray(out_proj_w.T).astype(ml_dtypes.bfloat16)  # [DI, D]
    convw_r = np.ascontiguousarray(
        conv_w.reshape(KC, 128, 4).transpose(1, 0, 2).reshape(128, KC * 4))
    convb_r = np.ascontiguousarray(conv_b.reshape(KC, 128).T)
    gateb_r = np.ascontiguousarray(gate_b.reshape(KC, 128).T)
    normb_r = np.ascontiguousarray(norm_b.reshape(KD, 128).T)

    in_maps = []
    for core in range(8):
        b, j = core // 4, core % 4
        xs = np.zeros((NT, D), np.float32)
        start = j * CHUNK - W
        mask = np.ones((1, NT), np.float32)
        if j == 0:
            xs[W:] = x[b, 0:CHUNK]
            mask[0, :W] = 0.0
        else:
            xs[:] = x[b, start:start + NT]
        in_maps.append({
            "x": np.ascontiguousarray(xs), "w1x": w1xT, "w1z": w1zT,
            "gw": gw8, "opw": opT, "convw": convw_r, "convb": convb_r,
            "gateb": gateb_r, "normb": normb_r,
            "mask": mask.astype(ml_dtypes.bfloat16),
        })
    return in_maps


def kernel(x, norm_w, norm_b, in_proj_w, conv_w, conv_b, gate_w, gate_b,
           out_proj_w, _trace=False, _collect=None):
    x = np.asarray(x, np.float32)
    if "nc" not in _cache:
        _cache["nc"] = _build()
    nc = _cache["nc"]
    in_maps = _prep_host(
        x, np.asarray(norm_w, np.float32), np.asarray(norm_b, np.float32),
        np.asarray(in_proj_w, np.float32), np.asarray(conv_w, np.float32),
        np.asarray(conv_b, np.float32), np.asarray(gate_w, np.float32),
        np.asarray(gate_b, np.float32), np.asarray(out_proj_w, np.float32))
    res = run_bass_kernel_spmd(nc, in_maps, core_ids=list(range(8)), trace=_trace)
    if _collect is not None:
        _collect.append(res)
    out = np.empty((B, L, D), np.float32)
    for core in range(8):
        b, j = core // 4, core % 4
        out[b, j * CHUNK:(j + 1) * CHUNK] = res.results[core]["out"]
    return out


# revision 10
# speedup vs baseline: 1.6461x; 1.6461x over previous
"""GatedLinearRecurrence Trainium2 kernel (8-core SPMD, Bass/Tile).

Strategy: shard (batch=2) x (4 sequence chunks of 1024 tokens) across 8 cores.
Each core processes 1152 tokens: a 128-token warm-up window (re-computed
redundantly; the recurrence decay makes carry-in truncation error ~1e-24)
followed by its 1024 "main" tokens.  No collectives needed.

Per-core pipeline (channels-on-partitions, tokens-on-free layout):
  LN(x) [t,d] -> PE-transpose (bf16) -> x̂T [d,t] -> in_proj (bf16 matmul)
  -> causal depthwise conv (bf16, 4 shifted tensor_scalar ops) -> silu -> mask
  -> fp8 quantized copy -> gate matmul (fp8e4m3 DoubleRow, 2 k-tiles/instr)
  -> sigmoid (dequant via activation scale) -> b=-(1-a)*xc
  -> tensor_tensor_scan (h=-h, fp32 carry) -> y*silu(z) -> out_proj (bf16)
  -> residual subtract -> out [t,dm].

Dtype plan (validated numerically: rel err ~5e-3 vs 2e-2 budget):
  - gate matmul fp8e4m3 both operands (weights pre-scaled x512 on host,
    activations x16 in-kernel; sigmoid activation scale 1/8192 dequantizes).
    DoubleRow perf mode halves PE time for this largest matmul.
  - everything else bf16 (same PE rate as f32r, half the DMA + DVE cost).
  - silu(z) stays resident in SBUF (no HBM scratch roundtrip).

The sign trick: scan data1 = (a-1)*x_conv = -b gives -h; -h*silu(z) = -yg;
out = x - matmul(-yg) = x + proj(yg).
"""
import sys

for p in ("/opt/trn_rl_repo", "/root/.axon_site/_ro/trn_rl_repo"):
    if p not in sys.path:
        sys.path.insert(0, p)

import numpy as np
import ml_dtypes

import concourse.bass as bass
import concourse.bacc as bacc
import concourse.tile as tile
import concourse.mybir as mybir
from concourse.bass_utils import run_bass_kernel_spmd
from concourse.masks import make_identity

F32 = mybir.dt.float32
BF16 = mybir.dt.bfloat16
F8 = mybir.dt.float8e4
AF = mybir.ActivationFunctionType
OP = mybir.AluOpType
DR = mybir.MatmulPerfMode.DoubleRow

B, L, D = 2, 4096, 1024
DI = 2048            # d_inner
NT = 1152            # tokens per core (128 warm-up + 1024 main)
W = 128              # warm-up tokens
CHUNK = 1024
NTT = NT // 128      # 9 token tiles
KD = D // 128        # 8 k-tiles over d_model
KC = DI // 128       # 16 k-tiles over d_inner
TC = 384             # matmul N chunk (3 per core)
NTC = NT // TC
EPS = 1e-5
S_GW = 512.0         # host-side fp8 scale on gate weights
DEQ = 1.0 / S_GW     # x_conv is quantized to fp8 unscaled (values < 10)
NB = 512             # out_proj column half width

_cache = {}


def _build():
    nc = bacc.Bacc(None, target_bir_lowering=False)

    x_h = nc.dram_tensor("x", [NT, D], F32, kind="ExternalInput")
    w1x_h = nc.dram_tensor("w1x", [D, DI], BF16, kind="ExternalInput")
    w1z_h = nc.dram_tensor("w1z", [D, DI], BF16, kind="ExternalInput")
    gw_h = nc.dram_tensor("gw", [DI, DI], F8, kind="ExternalInput")
    op_h = nc.dram_tensor("opw", [DI, D], BF16, kind="ExternalInput")
    convw_h = nc.dram_tensor("convw", [128, KC * 4], F32, kind="ExternalInput")
    convb_h = nc.dram_tensor("convb", [128, KC], F32, kind="ExternalInput")
    gateb_h = nc.dram_tensor("gateb", [128, KC], F32, kind="ExternalInput")
    normb_h = nc.dram_tensor("normb", [128, KD], F32, kind="ExternalInput")
    mask_h = nc.dram_tensor("mask", [1, NT], BF16, kind="ExternalInput")
    out_h = nc.dram_tensor("out", [CHUNK, D], F32, kind="ExternalOutput")

    with tile.TileContext(nc) as tc:
        with tc.tile_pool(name="consts", bufs=1) as consts:

            ident = consts.tile([128, 128], BF16, name="ident")
            make_identity(nc, ident)
            mask_sb = consts.tile([128, W], BF16, name="mask_sb")
            nc.gpsimd.dma_start(
                out=mask_sb,
                in_=bass.AP(tensor=mask_h, offset=0, ap=[[0, 128], [1, W]]),
            )
            convw = consts.tile([128, KC * 4], F32, name="convw")
            nc.gpsimd.dma_start(out=convw, in_=convw_h.ap())
            convb = consts.tile([128, KC], F32, name="convb")
            nc.gpsimd.dma_start(out=convb, in_=convb_h.ap())
            gateb = consts.tile([128, KC], F32, name="gateb")
            nc.gpsimd.dma_start(out=gateb, in_=gateb_h.ap())
            normb = consts.tile([128, KD], F32, name="normb")
            nc.gpsimd.dma_start(out=normb, in_=normb_h.ap())
            eps_t = consts.tile([128, 1], F32, name="eps_t")
            nc.vector.memset(eps_t, EPS)
            zeros_bf = consts.tile([128, NT], BF16, name="zeros_bf")
            nc.vector.memset(zeros_bf, 0.0)

            with tc.tile_pool(name="xcp", bufs=1) as xcp, \
                 tc.tile_pool(name="zsp", bufs=1) as zsp:
                xc = [xcp.tile([128, NT], BF16, name=f"xct{e}") for e in range(KC)]
                # fp8 pair tiles for the DoubleRow gate matmul: pair j holds
                # channel tiles 2j (dim1=0) and 2j+1 (dim1=1), scaled by S_XC
                xc8 = [xcp.tile([128, 2, NT], F8, name=f"xc8_{j}")
                       for j in range(KC // 2)]
                zs = [zsp.tile([128, CHUNK], BF16, name=f"zs{e}") for e in range(KC)]

                # ---- S1-S3: LN, transpose, in_proj (x & z), conv, silu ----
                with tc.tile_pool(name="xT", bufs=1) as xTp, \
                     tc.tile_pool(name="s1roll", bufs=2) as s1r, \
                     tc.tile_pool(name="stat", bufs=4) as stp, \
                     tc.tile_pool(name="w1s", bufs=3) as ws, \
                     tc.tile_pool(name="psmm", bufs=4, space="PSUM") as psmm, \
                     tc.tile_pool(name="psz", bufs=2, space="PSUM") as pszp, \
                     tc.tile_pool(name="pstr", bufs=2, space="PSUM") as pstr:

                    # x-hat-T chunk tiles [d-tile][t-chunk]: finer deps, so
                    # the first in_proj matmuls start after 3 LN iterations.
                    xT = [[xTp.tile([128, TC], BF16, name=f"xTt{d_}_{c_}")
                           for c_ in range(NTC)] for d_ in range(KD)]

                    # prefetch the first in_proj weight tiles on the (idle)
                    # scalar queue so the PE isn't stuck behind the 4.6MB of
                    # x token-tile loads on the sync queue
                    wts, xins = {}, {}
                    for et in range(4):
                        wt = ws.tile([128, KD, 128], BF16, tag="w1",
                                     bufs=8, name=f"wt{et}")
                        nc.scalar.dma_start(
                            out=wt,
                            in_=w1x_h.ap()[:, et * 128:(et + 1) * 128]
                            .rearrange("(kt p) e -> p kt e", p=128))
                        wts[et] = wt

                    for it in range(NTT):
                        tc3, col = it // 3, (it % 3) * 128
                        xt = s1r.tile([128, D], F32, tag="xt", bufs=3, name="xt")
                        nc.sync.dma_start(out=xt, in_=x_h.ap()[it * 128:(it + 1) * 128, :])
                        stats = stp.tile([128, 2, 6], F32, tag="stats", name="stats")
                        nc.vector.bn_stats(out=stats[:, 0, :], in_=xt[:, 0:512])
                        nc.vector.bn_stats(out=stats[:, 1, :], in_=xt[:, 512:1024])
                        mv = stp.tile([128, 2], F32, tag="mv", name="mv")
                        nc.vector.bn_aggr(out=mv, in_=stats)
                        rstd = stp.tile([128, 1], F32, tag="rstd", name="rstd")
                        nc.scalar.activation(out=rstd, in_=mv[:, 1:2], func=AF.Sqrt,
                                             bias=eps_t, scale=1.0)
                        nc.vector.reciprocal(out=rstd, in_=rstd)
                        xtn = s1r.tile([128, D], BF16, tag="xtn", bufs=3, name="xtn")
                        nc.vector.tensor_scalar(out=xtn, in0=xt, scalar1=mv[:, 0:1],
                                                scalar2=rstd, op0=OP.subtract, op1=OP.mult)
                        for d_ in range(KD):
                            pst = pstr.tile([128, 128], BF16, tag="tr", name="pst")
                            nc.tensor.transpose(pst, xtn[:, d_ * 128:(d_ + 1) * 128], ident)
                            # evac + norm_b on the Scalar engine
                            nc.scalar.activation(
                                out=xT[d_][tc3][:, col:col + 128], in_=pst,
                                func=AF.Identity, bias=normb[:, d_:d_ + 1], scale=1.0)

                    # in_proj x-half + conv + silu + warm-up mask + fp8 copy.
                    # The first 4 ets interleave their t-chunks so the PE never
                    # waits on layernorm chunks still in flight.
                    NW = 4
                    order = [(e, c) for c in range(NTC) for e in range(NW)]
                    order += [(e, c) for e in range(NW, KC) for c in range(NTC)]

                    def s2_chain(et, tc3):
                        if tc3 == 0:
                            if et not in wts:
                                wt = ws.tile([128, KD, 128], BF16, tag="w1",
                                             bufs=8, name=f"wt{et}")
                                nc.sync.dma_start(
                                    out=wt,
                                    in_=w1x_h.ap()[:, et * 128:(et + 1) * 128]
                                    .rearrange("(kt p) e -> p kt e", p=128))
                                wts[et] = wt
                            xin = s1r.tile([128, NT + 3], BF16, tag="xin",
                                           bufs=NW + 2, name=f"xin{et}")
                            nc.vector.memset(xin[:, 0:3], 0.0)
                            xins[et] = xin
                        ps = psmm.tile([128, TC], F32, tag="mm", name="ps")
                        for kt in range(KD):
                            nc.tensor.matmul(
                                ps, wts[et][:, kt, :], xT[kt][tc3],
                                start=(kt == 0), stop=(kt == KD - 1))
                        nc.scalar.copy(
                            out=xins[et][:, 3 + tc3 * TC: 3 + (tc3 + 1) * TC], in_=ps)
                        if tc3 == NTC - 1:
                            wts.pop(et)
                            xin = xins.pop(et)
                            tmp = s1r.tile([128, NT], BF16, tag="ctmp", name="ctmp")
                            for k in range(4):
                                nc.vector.scalar_tensor_tensor(
                                    out=tmp, in0=xin[:, k:k + NT],
                                    scalar=convw[:, et * 4 + k:et * 4 + k + 1],
                                    in1=zeros_bf if k == 0 else tmp,
                                    op0=OP.mult, op1=OP.add)
                            nc.scalar.activation(out=xc[et], in_=tmp, func=AF.Silu,
                                                 bias=convb[:, et:et + 1], scale=1.0)
                            # fp8 copy for the DoubleRow gate matmul (second
                            # silu on the scalar engine; unmasked is fine --
                            # the warm-up mask only needs to kill b, which is
                            # built from the bf16 xc)
                            nc.scalar.activation(
                                out=xc8[et // 2][:, et % 2, :], in_=tmp,
                                func=AF.Silu, bias=convb[:, et:et + 1], scale=1.0)
                            # mask is non-unit only on the warm-up columns
                            nc.vector.tensor_mul(
                                xc[et][:, 0:W], xc[et][:, 0:W], mask_sb)

                    for et, tc3 in order:
                        s2_chain(et, tc3)

                    # in_proj z-half + silu -> SBUF-resident zs (skip warm-up:
                    # the first chunk starts at token W)
                    for et in range(KC):
                        wt = ws.tile([128, KD, 128], BF16, tag="w1", bufs=8, name="wtz")
                        nc.sync.dma_start(
                            out=wt,
                            in_=w1z_h.ap()[:, et * 128:(et + 1) * 128]
                            .rearrange("(kt p) e -> p kt e", p=128))
                        for tc3 in range(NTC):
                            lo = max(tc3 * TC, W)
                            n = (tc3 + 1) * TC - lo
                            ps = pszp.tile([128, TC], F32, tag="mmz", name="psz")
                            for kt in range(KD):
                                nc.tensor.matmul(
                                    ps[:, 0:n], wt[:, kt, :], xT[kt][tc3][:, lo - tc3 * TC:TC],
                                    start=(kt == 0), stop=(kt == KD - 1))
                            nc.scalar.activation(
                                out=zs[et][:, lo - W:(tc3 + 1) * TC - W],
                                in_=ps[:, 0:n], func=AF.Silu)

                    # timing probe (result unused): measure the DVE bf16->fp8
                    # copy rate for future offload decisions
                    prb8 = s1r.tile([128, NT], F8, tag="prb8", name="probe_dve8")
                    nc.vector.tensor_copy(prb8, xc[2])

                # ---- S4-S6: gate matmul (fp8 DoubleRow), sigmoid, chunked
                # scan, y*silu(z).  y is kept in SBUF as chunk tiles whose
                # column slices feed out_proj directly as stationary operands.
                with tc.tile_pool(name="yp", bufs=1) as yp, \
                     tc.tile_pool(name="opwp", bufs=1) as opp:
                    ych = [[None] * NTC for _ in range(KC)]
                    # full out_proj weight prefetch (bf16, 4.2MB) on the
                    # scalar queue while the PE runs the gate matmuls
                    opw_sb = opp.tile([128, KC, D], BF16, name="opw_sb")
                    nc.scalar.dma_start(
                        out=opw_sb,
                        in_=op_h.ap().rearrange("(kt p) d -> p kt d", p=128))

                    with tc.tile_pool(name="gws", bufs=3) as gs, \
                         tc.tile_pool(name="ach", bufs=6) as ayp, \
                         tc.tile_pool(name="s6roll", bufs=6) as s6r, \
                         tc.tile_pool(name="psg", bufs=4, space="PSUM") as psg:

                        for et in range(KC):
                            gt = gs.tile([128, KC, 128], F8, tag="gw", name="gt")
                            nc.sync.dma_start(
                                out=gt,
                                in_=gw_h.ap()[:, et * 128:(et + 1) * 128]
                                .rearrange("(kt p) e -> p kt e", p=128))
                            prev_y = None
                            for tc3 in range(NTC):
                                a_t = ayp.tile([128, TC], BF16, tag="ach", name="ach")
                                ps = psg.tile([128, TC], F32, tag="mm", name="psgt")
                                for j in range(KC // 2):
                                    nc.tensor.matmul(
                                        ps, gt[:, 2 * j:2 * j + 2, :],
                                        xc8[j][:, :, tc3 * TC:(tc3 + 1) * TC],
                                        start=(j == 0), stop=(j == KC // 2 - 1),
                                        perf_mode=DR)
                                nc.scalar.activation(
                                    out=a_t, in_=ps,
                                    func=AF.Sigmoid, bias=gateb[:, et:et + 1],
                                    scale=DEQ)
                                bt = s6r.tile([128, TC], BF16, tag="bt", name="bt")
                                nc.vector.scalar_tensor_tensor(
                                    out=bt, in0=a_t, scalar=1.0,
                                    in1=xc[et][:, tc3 * TC:(tc3 + 1) * TC],
                                    op0=OP.subtract, op1=OP.mult)
                                y_t = yp.tile([128, TC], BF16, name=f"y{et}_{tc3}")
                                init = 0.0 if tc3 == 0 else prev_y[:, TC - 1:TC]
                                nc.vector.tensor_tensor_scan(
                                    out=y_t, data0=a_t, data1=bt, initial=init,
                                    op0=OP.mult, op1=OP.add)
                                ych[et][tc3] = y_t
                                prev_y = y_t
                            # -yg: multiply after the carry chain is complete
                            for tc3 in range(NTC):
                                lo = max(tc3 * TC, W)
                                nc.vector.tensor_mul(
                                    ych[et][tc3][:, lo - tc3 * TC:TC],
                                    ych[et][tc3][:, lo - tc3 * TC:TC],
                                    zs[et][:, lo - W:(tc3 + 1) * TC - W])

                    # ---- S7: out_proj + residual.  y chunk slices are the
                    # stationary operands; 4 passes of (column half x 4 token
                    # blocks), 4 rolling PSUM banks so pass n+1 matmuls
                    # overlap pass n evac. ----
                    def yslice(kt, tb):
                        col = W + tb * 128          # absolute column in [0, NT)
                        tc3, off = col // TC, col % TC
                        return ych[kt][tc3][:, off:off + 128]

                    with tc.tile_pool(name="s7roll", bufs=6) as s7r, \
                         tc.tile_pool(name="s7res", bufs=8) as s7x, \
                         tc.tile_pool(name="psop", bufs=4, space="PSUM") as psop:
                        for nb in range(2):
                            for tbg in range(2):
                                tbs = [tbg * 4 + i for i in range(4)]
                                xres = {}
                                for tb in tbs:
                                    xres[tb] = s7x.tile([128, NB], F32, tag="xres",
                                                        name=f"xres{nb}_{tb}")
                                    nc.scalar.dma_start(
                                        out=xres[tb],
                                        in_=x_h.ap()[W + tb * 128:W + (tb + 1) * 128,
                                                     nb * NB:(nb + 1) * NB])
                                pss = {tb: psop.tile([128, NB], F32, tag="op",
                                                     name=f"pso{tb}") for tb in tbs}
                                for kt in range(KC):
                                    for tb in tbs:
                                        nc.tensor.matmul(
                                            pss[tb], yslice(kt, tb),
                                            opw_sb[:, kt, nb * NB:(nb + 1) * NB],
                                            start=(kt == 0), stop=(kt == KC - 1))
                                for tb in tbs:
                                    oh = s7r.tile([128, NB], F32, tag="oh", name="oh")
                                    nc.vector.tensor_sub(oh, xres[tb], pss[tb])
                                    nc.sync.dma_start(
                                        out=out_h.ap()[tb * 128:(tb + 1) * 128,
                                                       nb * NB:(nb + 1) * NB],
                                        in_=oh)

    nc.compile()
    return nc


def _prep_host(x, norm_w, norm_b, in_proj_w, conv_w, conv_b, gate_w, gate_b,
               out_proj_w):
    w1 = (in_proj_w * norm_w[None, :]).astype(np.float32)
    w1xT = np.ascontiguousarray(w1[:DI].T).astype(ml_dtypes.bfloat16)   # [D, DI]
    w1zT = np.ascontiguousarray(w1[DI:].T).astype(ml_dtypes.bfloat16)   # [D, DI]
    gwT = np.ascontiguousarray(gate_w.T)             # [DI, DI]
    gw8 = np.clip(gwT * S_GW, -240, 240).astype(ml_dtypes.float8_e4m3)
    opT = np.ascontiguousarray(out_proj_w.T).astype(ml_dtypes.bfloat16)  # [DI, D]
    convw_r = np.ascontiguousarray(
        conv_w.reshape(KC, 128, 4).transpose(1, 0, 2).reshape(128, KC * 4))
    convb_r = np.ascontiguousarray(conv_b.reshape(KC, 128).T)
    gateb_r = np.ascontiguousarray(gate_b.reshape(KC, 128).T)
    normb_r = np.ascontiguousarray(norm_b.reshape(KD, 128).T)

    in_maps = []
    for core in range(8):
        b, j = core // 4, core % 4
        xs = np.zeros((NT, D), np.float32)
        start = j * CHUNK - W
        mask = np.ones((1, NT), np.float32)
        if j == 0:
            xs[W:] = x[b, 0:CHUNK]
            mask[0, :W] = 0.0
        else:
            xs[:] = x[b, start:start + NT]
        in_maps.append({
            "x": np.ascontiguousarray(xs), "w1x": w1xT, "w1z": w1zT,
            "gw": gw8, "opw": opT, "convw": convw_r, "convb": convb_r,
            "gateb": gateb_r, "normb": normb_r,
            "mask": mask.astype(ml_dtypes.bfloat16),
        })
    return in_maps


def kernel(x, norm_w, norm_b, in_proj_w, conv_w, conv_b, gate_w, gate_b,
           out_proj_w, _trace=False, _collect=None):
    x = np.asarray(x, np.float32)
    if "nc" not in _cache:
        _cache["nc"] = _build()
    nc = _cache["nc"]
    in_maps = _prep_host(
        x, np.asarray(norm_w, np.float32), np.asarray(norm_b, np.float32),
        np.asarray(in_proj_w, np.float32), np.asarray(conv_w, np.float32),
        np.asarray(conv_b, np.float32), np.asarray(gate_w, np.float32),
        np.asarray(gate_b, np.float32), np.asarray(out_proj_w, np.float32))
    res = run_bass_kernel_spmd(nc, in_maps, core_ids=list(range(8)), trace=_trace)
    if _collect is not None:
        _collect.append(res)
    out = np.empty((B, L, D), np.float32)
    for core in range(8):
        b, j = core // 4, core % 4
        out[b, j * CHUNK:(j + 1) * CHUNK] = res.results[core]["out"]
    return out


# revision 21
# speedup vs baseline: 1.6747x; 1.0174x over previous
"""GatedLinearRecurrence Trainium2 kernel (8-core SPMD, Bass/Tile).

Strategy: shard (batch=2) x (4 sequence chunks of 1024 tokens) across 8 cores.
Each core processes 1152 tokens: a 128-token warm-up window (re-computed
redundantly; the recurrence decay makes carry-in truncation error ~1e-24)
followed by its 1024 "main" tokens.  No collectives needed.

Per-core pipeline (channels-on-partitions, tokens-on-free layout):
  LN(x) [t,d] -> PE-transpose (bf16) -> x̂T [d,t] -> in_proj (bf16 matmul)
  -> causal depthwise conv (bf16, 4 shifted tensor_scalar ops) -> silu -> mask
  -> fp8 quantized copy -> gate matmul (fp8e4m3 DoubleRow, 2 k-tiles/instr)
  -> sigmoid (dequant via activation scale) -> b=-(1-a)*xc
  -> tensor_tensor_scan (h=-h, fp32 carry) -> y*silu(z) -> out_proj (bf16)
  -> residual subtract -> out [t,dm].

Dtype plan (validated numerically: rel err ~5e-3 vs 2e-2 budget):
  - gate matmul fp8e4m3 both operands (weights pre-scaled x512 on host,
    activations x16 in-kernel; sigmoid activation scale 1/8192 dequantizes).
    DoubleRow perf mode halves PE time for this largest matmul.
  - everything else bf16 (same PE rate as f32r, half the DMA + DVE cost).
  - silu(z) stays resident in SBUF (no HBM scratch roundtrip).

The sign trick: scan data1 = (a-1)*x_conv = -b gives -h; -h*silu(z) = -yg;
out = x - matmul(-yg) = x + proj(yg).
"""
import sys

for p in ("/opt/trn_rl_repo", "/root/.axon_site/_ro/trn_rl_repo"):
    if p not in sys.path:
        sys.path.insert(0, p)

import numpy as np
import ml_dtypes

import concourse.bass as bass
import concourse.bacc as bacc
import concourse.tile as tile
import concourse.mybir as mybir
from concourse.bass_utils import run_bass_kernel_spmd
from concourse.masks import make_identity

F32 = mybir.dt.float32
BF16 = mybir.dt.bfloat16
F8 = mybir.dt.float8e4
AF = mybir.ActivationFunctionType
OP = mybir.AluOpType
DR = mybir.MatmulPerfMode.DoubleRow

B, L, D = 2, 4096, 1024
DI = 2048            # d_inner
NT = 1152            # tokens per core (128 warm-up + 1024 main)
W = 128              # warm-up tokens
CHUNK = 1024
NTT = NT // 128      # 9 token tiles
KD = D // 128        # 8 k-tiles over d_model
KC = DI // 128       # 16 k-tiles over d_inner
TC = 384             # matmul N chunk (3 per core)
NTC = NT // TC
EPS = 1e-5
S_GW = 512.0         # host-side fp8 scale on gate weights
DEQ = 1.0 / S_GW     # x_conv is quantized to fp8 unscaled (values < 10)
NB = 512             # out_proj column half width

_cache = {}


def _build():
    nc = bacc.Bacc(None, target_bir_lowering=False)

    x_h = nc.dram_tensor("x", [NT, D], F32, kind="ExternalInput")
    w1x_h = nc.dram_tensor("w1x", [D, DI], BF16, kind="ExternalInput")
    w1z_h = nc.dram_tensor("w1z8", [D, DI], F8, kind="ExternalInput")
    gw_h = nc.dram_tensor("gw", [DI, DI], F8, kind="ExternalInput")
    op_h = nc.dram_tensor("opw", [DI, D], BF16, kind="ExternalInput")
    convw_h = nc.dram_tensor("convw", [128, KC * 4], BF16, kind="ExternalInput")
    convb_h = nc.dram_tensor("convb", [128, KC], F32, kind="ExternalInput")
    gateb_h = nc.dram_tensor("gateb", [128, KC], F32, kind="ExternalInput")
    normb_h = nc.dram_tensor("normb", [128, KD], F32, kind="ExternalInput")
    mask_h = nc.dram_tensor("mask", [1, NT], BF16, kind="ExternalInput")
    out_h = nc.dram_tensor("out", [CHUNK, D], F32, kind="ExternalOutput")

    with tile.TileContext(nc) as tc:
        with tc.tile_pool(name="consts", bufs=1) as consts:

            ident = consts.tile([128, 128], BF16, name="ident")
            make_identity(nc, ident)
            mask_sb = consts.tile([128, W], BF16, name="mask_sb")
            nc.gpsimd.dma_start(
                out=mask_sb,
                in_=bass.AP(tensor=mask_h, offset=0, ap=[[0, 128], [1, W]]),
            )
            convw = consts.tile([128, KC * 4], BF16, name="convw")
            nc.gpsimd.dma_start(out=convw, in_=convw_h.ap())
            convb = consts.tile([128, KC], F32, name="convb")
            nc.gpsimd.dma_start(out=convb, in_=convb_h.ap())
            gateb = consts.tile([128, KC], F32, name="gateb")
            nc.gpsimd.dma_start(out=gateb, in_=gateb_h.ap())
            normb = consts.tile([128, KD], F32, name="normb")
            nc.gpsimd.dma_start(out=normb, in_=normb_h.ap())
            eps_t = consts.tile([128, 1], F32, name="eps_t")
            nc.vector.memset(eps_t, EPS)
            # pre-warm the activation function tables so the first LN tile
            # doesn't eat a 1.3us ACT_TABLE_LOAD on the critical path
            warm = consts.tile([128, 4], F32, name="warm")
            nc.scalar.activation(out=warm[:, 0:1], in_=eps_t, func=AF.Sqrt)
            nc.scalar.activation(out=warm[:, 1:2], in_=eps_t, func=AF.Identity)
            nc.scalar.activation(out=warm[:, 2:3], in_=eps_t, func=AF.Silu)
            nc.scalar.activation(out=warm[:, 3:4], in_=eps_t, func=AF.Sigmoid)
            zeros_bf = consts.tile([128, NT], BF16, name="zeros_bf")
            nc.vector.memset(zeros_bf, 0.0)

            with tc.tile_pool(name="xcp", bufs=1) as xcp, \
                 tc.tile_pool(name="zsp", bufs=1) as zsp:
                xc = [xcp.tile([128, NT], BF16, name=f"xct{e}") for e in range(KC)]
                # fp8 pair tiles for the DoubleRow gate matmul: pair j holds
                # channel tiles 2j (dim1=0) and 2j+1 (dim1=1), scaled by S_XC
                xc8 = [xcp.tile([128, 2, NT], F8, name=f"xc8_{j}")
                       for j in range(KC // 2)]
                zs = [zsp.tile([128, CHUNK], BF16, name=f"zs{e}") for e in range(KC)]

                # ---- S1-S3: LN, transpose, in_proj (x & z), conv, silu ----
                with tc.tile_pool(name="xT", bufs=1) as xTp, \
                     tc.tile_pool(name="s1roll", bufs=2) as s1r, \
                     tc.tile_pool(name="stat", bufs=4) as stp, \
                     tc.tile_pool(name="w1s", bufs=3) as ws, \
                     tc.tile_pool(name="psmm", bufs=4, space="PSUM") as psmm, \
                     tc.tile_pool(name="psz", bufs=2, space="PSUM") as pszp, \
                     tc.tile_pool(name="pstr", bufs=2, space="PSUM") as pstr:

                    # x-hat-T chunk tiles [d-tile][t-chunk]: finer deps, so
                    # the first in_proj matmuls start after 3 LN iterations.
                    xT = [[xTp.tile([128, TC], BF16, name=f"xTt{d_}_{c_}")
                           for c_ in range(NTC)] for d_ in range(KD)]
                    # fp8 x-hat pair tiles for the DoubleRow z matmul: pair dp
                    # holds d-tiles 2dp/2dp+1, scaled x16
                    xT8 = [[xTp.tile([128, 2, TC], F8, name=f"xT8_{dp}_{c_}")
                            for c_ in range(NTC)] for dp in range(KD // 2)]

                    # prefetch the first in_proj weight tiles on the (idle)
                    # scalar queue so the PE isn't stuck behind the 4.6MB of
                    # x token-tile loads on the sync queue
                    wts, xins = {}, {}
                    for et in range(4):
                        wt = ws.tile([128, KD, 128], BF16, tag="w1",
                                     bufs=8, name=f"wt{et}")
                        nc.scalar.dma_start(
                            out=wt,
                            in_=w1x_h.ap()[:, et * 128:(et + 1) * 128]
                            .rearrange("(kt p) e -> p kt e", p=128))
                        wts[et] = wt

                    def ln_iter(it):
                        tc3, col = it // 3, (it % 3) * 128
                        xt = s1r.tile([128, D], F32, tag="xt", bufs=3, name="xt")
                        if it == 0:
                            # split so the first bn_stats starts a half earlier
                            nc.sync.dma_start(out=xt[:, 0:512],
                                              in_=x_h.ap()[0:128, 0:512])
                            nc.sync.dma_start(out=xt[:, 512:1024],
                                              in_=x_h.ap()[0:128, 512:1024])
                        else:
                            nc.sync.dma_start(
                                out=xt, in_=x_h.ap()[it * 128:(it + 1) * 128, :])
                        stats = stp.tile([128, 2, 6], F32, tag="stats", name="stats")
                        nc.vector.bn_stats(out=stats[:, 0, :], in_=xt[:, 0:512])
                        nc.vector.bn_stats(out=stats[:, 1, :], in_=xt[:, 512:1024])
                        mv = stp.tile([128, 2], F32, tag="mv", name="mv")
                        nc.vector.bn_aggr(out=mv, in_=stats)
                        rstd = stp.tile([128, 1], F32, tag="rstd", name="rstd")
                        nc.scalar.activation(out=rstd, in_=mv[:, 1:2], func=AF.Sqrt,
                                             bias=eps_t, scale=1.0)
                        nc.vector.reciprocal(out=rstd, in_=rstd)
                        xtn = s1r.tile([128, D], BF16, tag="xtn", bufs=3, name="xtn")
                        nc.vector.tensor_scalar(out=xtn, in0=xt, scalar1=mv[:, 0:1],
                                                scalar2=rstd, op0=OP.subtract, op1=OP.mult)
                        for d_ in range(KD):
                            pst = pstr.tile([128, 128], BF16, tag="tr", name="pst")
                            nc.tensor.transpose(pst, xtn[:, d_ * 128:(d_ + 1) * 128], ident)
                            # evac + norm_b on the Scalar engine
                            nc.scalar.activation(
                                out=xT[d_][tc3][:, col:col + 128], in_=pst,
                                func=AF.Identity, bias=normb[:, d_:d_ + 1], scale=1.0)
                        if it % 3 == 2:
                            # chunk complete: emit the fp8 x-hat pair tiles
                            # (xT already includes norm_b; just scale by 16)
                            for dp in range(KD // 2):
                                for i in range(2):
                                    nc.scalar.activation(
                                        out=xT8[dp][tc3][:, i, :],
                                        in_=xT[2 * dp + i][tc3], func=AF.Identity,
                                        scale=16.0)

                    # in_proj x-half + conv + silu + warm-up mask + fp8 copy.
                    # The first 4 ets interleave their t-chunks (issued inside
                    # the LN loop) so the PE never waits on layernorm chunks
                    # still in flight.
                    NW = 4

                    def s2_chain(et, tc3):
                        if tc3 == 0:
                            if et not in wts:
                                wt = ws.tile([128, KD, 128], BF16, tag="w1",
                                             bufs=8, name=f"wt{et}")
                                nc.sync.dma_start(
                                    out=wt,
                                    in_=w1x_h.ap()[:, et * 128:(et + 1) * 128]
                                    .rearrange("(kt p) e -> p kt e", p=128))
                                wts[et] = wt
                            xin = s1r.tile([128, NT + 3], BF16, tag="xin",
                                           bufs=NW + 2, name=f"xin{et}")
                            nc.vector.memset(xin[:, 0:3], 0.0)
                            xins[et] = xin
                        ps = psmm.tile([128, TC], F32, tag="mm", name="ps")
                        for kt in range(KD):
                            nc.tensor.matmul(
                                ps, wts[et][:, kt, :], xT[kt][tc3],
                                start=(kt == 0), stop=(kt == KD - 1))
                        nc.scalar.copy(
                            out=xins[et][:, 3 + tc3 * TC: 3 + (tc3 + 1) * TC], in_=ps)
                        if tc3 == NTC - 1:
                            wts.pop(et)
                            xin = xins.pop(et)
                            tmp = s1r.tile([128, NT], BF16, tag="ctmp", name="ctmp")
                            for k in range(4):
                                nc.vector.scalar_tensor_tensor(
                                    out=tmp, in0=xin[:, k:k + NT],
                                    scalar=convw[:, et * 4 + k:et * 4 + k + 1],
                                    in1=zeros_bf if k == 0 else tmp,
                                    op0=OP.mult, op1=OP.add)
                            nc.scalar.activation(out=xc[et], in_=tmp, func=AF.Silu,
                                                 bias=convb[:, et:et + 1], scale=1.0)
                            # fp8 copy for the DoubleRow gate matmul (second
                            # silu on the scalar engine; unmasked is fine --
                            # the warm-up mask only needs to kill b, which is
                            # built from the bf16 xc)
                            nc.scalar.activation(
                                out=xc8[et // 2][:, et % 2, :], in_=tmp,
                                func=AF.Silu, bias=convb[:, et:et + 1], scale=1.0)
                            # mask is non-unit only on the warm-up columns
                            nc.vector.tensor_mul(
                                xc[et][:, 0:W], xc[et][:, 0:W], mask_sb)

                    # issue: LN iterations with the first NW ets' in_proj
                    # chains interleaved per chunk, then the remaining ets
                    for it in range(NTT):
                        ln_iter(it)
                        if it % 3 == 2:
                            for e in range(NW):
                                s2_chain(e, it // 3)
                    for e in range(NW, KC):
                        for c in range(NTC):
                            s2_chain(e, c)

                    # in_proj z-half (fp8 DoubleRow) + silu -> SBUF-resident
                    # zs (skip warm-up: the first chunk starts at token W);
                    # the silu activation scale de-quantizes x16 * x512
                    for et in range(KC):
                        wt = ws.tile([128, KD, 128], F8, tag="wz8", bufs=4,
                                     name="wtz")
                        nc.sync.dma_start(
                            out=wt,
                            in_=w1z_h.ap()[:, et * 128:(et + 1) * 128]
                            .rearrange("(kt p) e -> p kt e", p=128))
                        for tc3 in range(NTC):
                            lo = max(tc3 * TC, W)
                            n = (tc3 + 1) * TC - lo
                            off = lo - tc3 * TC
                            ps = pszp.tile([128, TC], F32, tag="mmz", name="psz")
                            for m in range(KD // 2):
                                nc.tensor.matmul(
                                    ps[:, 0:n], wt[:, 2 * m:2 * m + 2, :],
                                    xT8[m][tc3][:, :, off:TC],
                                    start=(m == 0), stop=(m == KD // 2 - 1),
                                    perf_mode=DR)
                            nc.scalar.activation(
                                out=zs[et][:, lo - W:(tc3 + 1) * TC - W],
                                in_=ps[:, 0:n], func=AF.Silu,
                                scale=1.0 / (16.0 * S_GW))

                # ---- S4-S6: gate matmul (fp8 DoubleRow), sigmoid, chunked
                # scan, y*silu(z).  y is kept in SBUF as chunk tiles whose
                # column slices feed out_proj directly as stationary operands.
                with tc.tile_pool(name="yp", bufs=1) as yp, \
                     tc.tile_pool(name="opwp", bufs=1) as opp:
                    ych = [[None] * NTC for _ in range(KC)]
                    # full out_proj weight prefetch (bf16, 4.2MB) on the
                    # scalar queue while the PE runs the gate matmuls
                    opw_sb = opp.tile([128, KC, D], BF16, name="opw_sb")
                    nc.scalar.dma_start(
                        out=opw_sb,
                        in_=op_h.ap().rearrange("(kt p) d -> p kt d", p=128))

                    with tc.tile_pool(name="gws", bufs=3) as gs, \
                         tc.tile_pool(name="ach", bufs=6) as ayp, \
                         tc.tile_pool(name="s6roll", bufs=6) as s6r, \
                         tc.tile_pool(name="psg", bufs=4, space="PSUM") as psg:

                        for et in range(KC):
                            gt = gs.tile([128, KC, 128], F8, tag="gw", name="gt")
                            nc.sync.dma_start(
                                out=gt,
                                in_=gw_h.ap()[:, et * 128:(et + 1) * 128]
                                .rearrange("(kt p) e -> p kt e", p=128))
                            prev_y = None
                            for tc3 in range(NTC):
                                a_t = ayp.tile([128, TC], BF16, tag="ach", name="ach")
                                ps = psg.tile([128, TC], F32, tag="mm", name="psgt")
                                for j in range(KC // 2):
                                    nc.tensor.matmul(
                                        ps, gt[:, 2 * j:2 * j + 2, :],
                                        xc8[j][:, :, tc3 * TC:(tc3 + 1) * TC],
                                        start=(j == 0), stop=(j == KC // 2 - 1),
                                        perf_mode=DR)
                                nc.scalar.activation(
                                    out=a_t, in_=ps,
                                    func=AF.Sigmoid, bias=gateb[:, et:et + 1],
                                    scale=DEQ)
                                bt = s6r.tile([128, TC], BF16, tag="bt", name="bt")
                                nc.vector.scalar_tensor_tensor(
                                    out=bt, in0=a_t, scalar=1.0,
                                    in1=xc[et][:, tc3 * TC:(tc3 + 1) * TC],
                                    op0=OP.subtract, op1=OP.mult)
                                y_t = yp.tile([128, TC], BF16, name=f"y{et}_{tc3}")
                                init = 0.0 if tc3 == 0 else prev_y[:, TC - 1:TC]
                                nc.vector.tensor_tensor_scan(
                                    out=y_t, data0=a_t, data1=bt, initial=init,
                                    op0=OP.mult, op1=OP.add)
                                ych[et][tc3] = y_t
                                prev_y = y_t
                            # -yg: multiply after the carry chain is complete
                            for tc3 in range(NTC):
                                lo = max(tc3 * TC, W)
                                nc.vector.tensor_mul(
                                    ych[et][tc3][:, lo - tc3 * TC:TC],
                                    ych[et][tc3][:, lo - tc3 * TC:TC],
                                    zs[et][:, lo - W:(tc3 + 1) * TC - W])

                    # ---- S7: out_proj + residual.  y chunk slices are the
                    # stationary operands; 4 passes of (column half x 4 token
                    # blocks), 4 rolling PSUM banks so pass n+1 matmuls
                    # overlap pass n evac. ----
                    def yslice(kt, tb):
                        col = W + tb * 128          # absolute column in [0, NT)
                        tc3, off = col // TC, col % TC
                        return ych[kt][tc3][:, off:off + 128]

                    with tc.tile_pool(name="s7roll", bufs=6) as s7r, \
                         tc.tile_pool(name="s7res", bufs=8) as s7x, \
                         tc.tile_pool(name="psop", bufs=4, space="PSUM") as psop:
                        for nb in range(2):
                            for tbg in range(4):
                                tbs = [tbg * 2 + i for i in range(2)]
                                xres = {}
                                for tb in tbs:
                                    xres[tb] = s7x.tile([128, NB], F32, tag="xres",
                                                        name=f"xres{nb}_{tb}")
                                    nc.scalar.dma_start(
                                        out=xres[tb],
                                        in_=x_h.ap()[W + tb * 128:W + (tb + 1) * 128,
                                                     nb * NB:(nb + 1) * NB])
                                pss = {tb: psop.tile([128, NB], F32, tag="op",
                                                     name=f"pso{tb}") for tb in tbs}
                                for kt in range(KC):
                                    for tb in tbs:
                                        nc.tensor.matmul(
                                            pss[tb], yslice(kt, tb),
                                            opw_sb[:, kt, nb * NB:(nb + 1) * NB],
                                            start=(kt == 0), stop=(kt == KC - 1))
                                for tb in tbs:
                                    oh = s7r.tile([128, NB], F32, tag="oh", name="oh")
                                    nc.vector.tensor_sub(oh, xres[tb], pss[tb])
                                    nc.sync.dma_start(
                                        out=out_h.ap()[tb * 128:(tb + 1) * 128,
                                                       nb * NB:(nb + 1) * NB],
                                        in_=oh)

    nc.compile()
    return nc


def _prep_host(x, norm_w, norm_b, in_proj_w, conv_w, conv_b, gate_w, gate_b,
               out_proj_w):
    w1 = (in_proj_w * norm_w[None, :]).astype(np.float32)
    w1xT = np.ascontiguousarray(w1[:DI].T).astype(ml_dtypes.bfloat16)   # [D, DI]
    w1z8 = np.clip(np.ascontiguousarray(w1[DI:].T) * S_GW, -240,
                   240).astype(ml_dtypes.float8_e4m3)                   # [D, DI]
    gwT = np.ascontiguousarray(gate_w.T)             # [DI, DI]
    gw8 = np.clip(gwT * S_GW, -240, 240).astype(ml_dtypes.float8_e4m3)
    opT = np.ascontiguousarray(out_proj_w.T).astype(ml_dtypes.bfloat16)  # [DI, D]
    convw_r = np.ascontiguousarray(
        conv_w.reshape(KC, 128, 4).transpose(1, 0, 2).reshape(128, KC * 4)
    ).astype(ml_dtypes.bfloat16)
    convb_r = np.ascontiguousarray(conv_b.reshape(KC, 128).T)
    gateb_r = np.ascontiguousarray(gate_b.reshape(KC, 128).T)
    normb_r = np.ascontiguousarray(norm_b.reshape(KD, 128).T)

    in_maps = []
    for core in range(8):
        b, j = core // 4, core % 4
        xs = np.zeros((NT, D), np.float32)
        start = j * CHUNK - W
        mask = np.ones((1, NT), np.float32)
        if j == 0:
            xs[W:] = x[b, 0:CHUNK]
            mask[0, :W] = 0.0
        else:
            xs[:] = x[b, start:start + NT]
        in_maps.append({
            "x": np.ascontiguousarray(xs), "w1x": w1xT, "w1z8": w1z8,
            "gw": gw8, "opw": opT, "convw": convw_r, "convb": convb_r,
            "gateb": gateb_r, "normb": normb_r,
            "mask": mask.astype(ml_dtypes.bfloat16),
        })
    return in_maps


def kernel(x, norm_w, norm_b, in_proj_w, conv_w, conv_b, gate_w, gate_b,
           out_proj_w, _trace=False, _collect=None):
    x = np.asarray(x, np.float32)
    if "nc" not in _cache:
        _cache["nc"] = _build()
    nc = _cache["nc"]
    in_maps = _prep_host(
        x, np.asarray(norm_w, np.float32), np.asarray(norm_b, np.float32),
        np.asarray(in_proj_w, np.float32), np.asarray(conv_w, np.float32),
        np.asarray(conv_b, np.float32), np.asarray(gate_w, np.float32),
        np.asarray(gate_b, np.float32), np.asarray(out_proj_w, np.float32))
    res = run_bass_kernel_spmd(nc, in_maps, core_ids=list(range(8)), trace=_trace)
    if _collect is not None:
        _collect.append(res)
    out = np.empty((B, L, D), np.float32)
    for core in range(8):
        b, j = core // 4, core % 4
        out[b, j * CHUNK:(j + 1) * CHUNK] = res.results[core]["out"]
    return out


# revision 33
# speedup vs baseline: 1.6838x; 1.0055x over previous
"""GatedLinearRecurrence Trainium2 kernel (8-core SPMD, Bass/Tile).

Strategy: shard (batch=2) x (4 sequence chunks of 1024 tokens) across 8 cores.
Each core processes 1152 tokens: a 128-token warm-up window (re-computed
redundantly; the recurrence decay makes carry-in truncation error ~1e-24)
followed by its 1024 "main" tokens.  No collectives needed.

Per-core pipeline (channels-on-partitions, tokens-on-free layout):
  LN(x) [t,d] -> PE-transpose (bf16) -> x̂T [d,t] -> in_proj (bf16 matmul)
  -> causal depthwise conv (bf16, 4 shifted tensor_scalar ops) -> silu -> mask
  -> fp8 quantized copy -> gate matmul (fp8e4m3 DoubleRow, 2 k-tiles/instr)
  -> sigmoid (dequant via activation scale) -> b=-(1-a)*xc
  -> tensor_tensor_scan (h=-h, fp32 carry) -> y*silu(z) -> out_proj (bf16)
  -> residual subtract -> out [t,dm].

Dtype plan (validated numerically: rel err ~5e-3 vs 2e-2 budget):
  - gate matmul fp8e4m3 both operands (weights pre-scaled x512 on host,
    activations x16 in-kernel; sigmoid activation scale 1/8192 dequantizes).
    DoubleRow perf mode halves PE time for this largest matmul.
  - everything else bf16 (same PE rate as f32r, half the DMA + DVE cost).
  - silu(z) stays resident in SBUF (no HBM scratch roundtrip).

The sign trick: scan data1 = (a-1)*x_conv = -b gives -h; -h*silu(z) = -yg;
out = x - matmul(-yg) = x + proj(yg).
"""
import sys

for p in ("/opt/trn_rl_repo", "/root/.axon_site/_ro/trn_rl_repo"):
    if p not in sys.path:
        sys.path.insert(0, p)

import numpy as np
import ml_dtypes

import concourse.bass as bass
import concourse.bacc as bacc
import concourse.tile as tile
import concourse.mybir as mybir
from concourse.bass_utils import run_bass_kernel_spmd
from concourse.masks import make_identity

F32 = mybir.dt.float32
BF16 = mybir.dt.bfloat16
F8 = mybir.dt.float8e4
AF = mybir.ActivationFunctionType
OP = mybir.AluOpType
DR = mybir.MatmulPerfMode.DoubleRow

B, L, D = 2, 4096, 1024
DI = 2048            # d_inner
NT = 1152            # tokens per core (128 warm-up + 1024 main)
W = 128              # warm-up tokens
CHUNK = 1024
NTT = NT // 128      # 9 token tiles
KD = D // 128        # 8 k-tiles over d_model
KC = DI // 128       # 16 k-tiles over d_inner
TC = 384             # matmul N chunk (3 per core)
NTC = NT // TC
EPS = 1e-5
S_GW = 512.0         # host-side fp8 scale on gate weights
DEQ = 1.0 / S_GW     # x_conv is quantized to fp8 unscaled (values < 10)
NB = 512             # out_proj column half width

_cache = {}


def _build():
    nc = bacc.Bacc(None, target_bir_lowering=False)

    x_h = nc.dram_tensor("x", [NT, D], F32, kind="ExternalInput")
    w1x_h = nc.dram_tensor("w1x", [D, DI], BF16, kind="ExternalInput")
    w1z_h = nc.dram_tensor("w1z8", [D, DI], F8, kind="ExternalInput")
    gw_h = nc.dram_tensor("gw", [DI, DI], F8, kind="ExternalInput")
    op_h = nc.dram_tensor("opw", [DI, D], BF16, kind="ExternalInput")
    convw_h = nc.dram_tensor("convw", [128, KC * 4], BF16, kind="ExternalInput")
    convb_h = nc.dram_tensor("convb", [128, KC], F32, kind="ExternalInput")
    gateb_h = nc.dram_tensor("gateb", [128, KC], F32, kind="ExternalInput")
    normb_h = nc.dram_tensor("normb", [128, KD], F32, kind="ExternalInput")
    mask_h = nc.dram_tensor("mask", [1, NT], BF16, kind="ExternalInput")
    out_h = nc.dram_tensor("out", [CHUNK, D], F32, kind="ExternalOutput")
    probe8_h = nc.dram_tensor("probe8", [128, NT], F8, kind="Internal")
    probeg_h = nc.dram_tensor("probeg", [128, NT], BF16, kind="Internal")

    with tile.TileContext(nc) as tc:
        with tc.tile_pool(name="consts", bufs=1) as consts:

            ident = consts.tile([128, 128], BF16, name="ident")
            make_identity(nc, ident)
            mask_sb = consts.tile([128, W], BF16, name="mask_sb")
            nc.gpsimd.dma_start(
                out=mask_sb,
                in_=bass.AP(tensor=mask_h, offset=0, ap=[[0, 128], [1, W]]),
            )
            convw = consts.tile([128, KC * 4], BF16, name="convw")
            nc.gpsimd.dma_start(out=convw, in_=convw_h.ap())
            convb = consts.tile([128, KC], F32, name="convb")
            nc.gpsimd.dma_start(out=convb, in_=convb_h.ap())
            gateb = consts.tile([128, KC], F32, name="gateb")
            nc.gpsimd.dma_start(out=gateb, in_=gateb_h.ap())
            normb = consts.tile([128, KD], F32, name="normb")
            nc.gpsimd.dma_start(out=normb, in_=normb_h.ap())
            eps_t = consts.tile([128, 1], F32, name="eps_t")
            nc.vector.memset(eps_t, EPS)
            # pre-warm the activation function tables so the first LN tile
            # doesn't eat a 1.3us ACT_TABLE_LOAD on the critical path
            warm = consts.tile([128, 4], F32, name="warm")
            nc.scalar.activation(out=warm[:, 0:1], in_=eps_t, func=AF.Sqrt)
            nc.scalar.activation(out=warm[:, 1:2], in_=eps_t, func=AF.Identity)
            nc.scalar.activation(out=warm[:, 2:3], in_=eps_t, func=AF.Silu)
            nc.scalar.activation(out=warm[:, 3:4], in_=eps_t, func=AF.Sigmoid)
            zeros_bf = consts.tile([128, NT], BF16, name="zeros_bf")
            nc.vector.memset(zeros_bf, 0.0)

            with tc.tile_pool(name="xcp", bufs=1) as xcp, \
                 tc.tile_pool(name="zsp", bufs=1) as zsp:
                xc = [xcp.tile([128, NT], BF16, name=f"xct{e}") for e in range(KC)]
                # fp8 pair tiles for the DoubleRow gate matmul: pair j holds
                # channel tiles 2j (dim1=0) and 2j+1 (dim1=1), scaled by S_XC
                xc8 = [xcp.tile([128, 2, NT], F8, name=f"xc8_{j}")
                       for j in range(KC // 2)]
                zs = [zsp.tile([128, CHUNK], BF16, name=f"zs{e}") for e in range(KC)]

                # ---- S1-S3: LN, transpose, in_proj (x & z), conv, silu ----
                NW = 6
                with tc.tile_pool(name="xT", bufs=1) as xTp, \
                     tc.tile_pool(name="s1roll", bufs=2) as s1r, \
                     tc.tile_pool(name="stat", bufs=4) as stp, \
                     tc.tile_pool(name="w1s", bufs=3) as ws:

                    # x-hat-T chunk tiles [d-tile][t-chunk]: finer deps, so
                    # the first in_proj matmuls start after 3 LN iterations.
                    xT = [[xTp.tile([128, TC], BF16, name=f"xTt{d_}_{c_}")
                           for c_ in range(NTC)] for d_ in range(KD)]
                    # fp8 x-hat pair tiles for the DoubleRow z matmul: pair dp
                    # holds d-tiles 2dp/2dp+1, scaled x16
                    xT8 = [[xTp.tile([128, 2, TC], F8, name=f"xT8_{dp}_{c_}")
                            for c_ in range(NTC)] for dp in range(KD // 2)]

                    # prefetch the first in_proj weight tiles on the (idle)
                    # gpsimd queue: DMA trigger instructions cost ~2us of
                    # engine time on the scalar queue, which would sit in
                    # front of the first layernorm Sqrt
                    psmm_cm = tc.tile_pool(name="psmm", bufs=4, space="PSUM")
                    pstr_cm = tc.tile_pool(name="pstr", bufs=2, space="PSUM")
                    psmm = psmm_cm.__enter__()
                    pstr = pstr_cm.__enter__()

                    wts, xins = {}, {}
                    for et in range(NW):
                        wt = ws.tile([128, KD, 128], BF16, tag="w1",
                                     bufs=10, name=f"wt{et}")
                        nc.gpsimd.dma_start(
                            out=wt,
                            in_=w1x_h.ap()[:, et * 128:(et + 1) * 128]
                            .rearrange("(kt p) e -> p kt e", p=128))
                        wts[et] = wt

                    def ln_iter(it):
                        tc3, col = it // 3, (it % 3) * 128
                        xt = s1r.tile([128, D], F32, tag="xt", bufs=3, name="xt")
                        if it == 0:
                            # split so the first bn_stats starts a half earlier
                            nc.sync.dma_start(out=xt[:, 0:512],
                                              in_=x_h.ap()[0:128, 0:512])
                            nc.sync.dma_start(out=xt[:, 512:1024],
                                              in_=x_h.ap()[0:128, 512:1024])
                        else:
                            nc.sync.dma_start(
                                out=xt, in_=x_h.ap()[it * 128:(it + 1) * 128, :])
                        stats = stp.tile([128, 2, 6], F32, tag="stats", name="stats")
                        nc.vector.bn_stats(out=stats[:, 0, :], in_=xt[:, 0:512])
                        nc.vector.bn_stats(out=stats[:, 1, :], in_=xt[:, 512:1024])
                        mv = stp.tile([128, 2], F32, tag="mv", name="mv")
                        nc.vector.bn_aggr(out=mv, in_=stats)
                        rstd = stp.tile([128, 1], F32, tag="rstd", name="rstd")
                        nc.scalar.activation(out=rstd, in_=mv[:, 1:2], func=AF.Sqrt,
                                             bias=eps_t, scale=1.0)
                        nc.vector.reciprocal(out=rstd, in_=rstd)
                        xtn = s1r.tile([128, D], BF16, tag="xtn", bufs=3, name="xtn")
                        nc.vector.tensor_scalar(out=xtn, in0=xt, scalar1=mv[:, 0:1],
                                                scalar2=rstd, op0=OP.subtract, op1=OP.mult)
                        for d_ in range(KD):
                            pst = pstr.tile([128, 128], BF16, tag="tr", name="pst")
                            nc.tensor.transpose(pst, xtn[:, d_ * 128:(d_ + 1) * 128], ident)
                            # evac + norm_b on the Scalar engine
                            nc.scalar.activation(
                                out=xT[d_][tc3][:, col:col + 128], in_=pst,
                                func=AF.Identity, bias=normb[:, d_:d_ + 1], scale=1.0)
                        if it % 3 == 2:
                            # chunk complete: emit the fp8 x-hat pair tiles
                            # (xT already includes norm_b; just scale by 16)
                            for dp in range(KD // 2):
                                for i in range(2):
                                    nc.scalar.activation(
                                        out=xT8[dp][tc3][:, i, :],
                                        in_=xT[2 * dp + i][tc3], func=AF.Identity,
                                        scale=16.0)

                    # in_proj x-half + conv + silu + warm-up mask + fp8 copy.
                    # The first NW ets interleave their t-chunks (issued
                    # inside the LN loop) so the PE never waits on layernorm
                    # chunks still in flight.
                    def s2_chain(et, tc3):
                        if tc3 == 0:
                            if et not in wts:
                                wt = ws.tile([128, KD, 128], BF16, tag="w1",
                                             bufs=10, name=f"wt{et}")
                                nc.sync.dma_start(
                                    out=wt,
                                    in_=w1x_h.ap()[:, et * 128:(et + 1) * 128]
                                    .rearrange("(kt p) e -> p kt e", p=128))
                                wts[et] = wt
                            xin = s1r.tile([128, NT + 3], BF16, tag="xin",
                                           bufs=10, name=f"xin{et}")
                            nc.vector.memset(xin[:, 0:3], 0.0)
                            xins[et] = xin
                        ps = psmm.tile([128, TC], F32, tag="mm", name="ps")
                        for kt in range(KD):
                            nc.tensor.matmul(
                                ps, wts[et][:, kt, :], xT[kt][tc3],
                                start=(kt == 0), stop=(kt == KD - 1))
                        nc.scalar.copy(
                            out=xins[et][:, 3 + tc3 * TC: 3 + (tc3 + 1) * TC], in_=ps)
                        if tc3 == NTC - 1:
                            wts.pop(et)
                            xin = xins.pop(et)
                            tmp = s1r.tile([128, NT], BF16, tag="ctmp", name="ctmp")
                            for k in range(4):
                                nc.vector.scalar_tensor_tensor(
                                    out=tmp, in0=xin[:, k:k + NT],
                                    scalar=convw[:, et * 4 + k:et * 4 + k + 1],
                                    in1=zeros_bf if k == 0 else tmp,
                                    op0=OP.mult, op1=OP.add)
                            nc.scalar.activation(out=xc[et], in_=tmp, func=AF.Silu,
                                                 bias=convb[:, et:et + 1], scale=1.0)
                            # fp8 copy for the DoubleRow gate matmul (second
                            # silu on the scalar engine; unmasked is fine --
                            # the warm-up mask only needs to kill b, which is
                            # built from the bf16 xc)
                            nc.scalar.activation(
                                out=xc8[et // 2][:, et % 2, :], in_=tmp,
                                func=AF.Silu, bias=convb[:, et:et + 1], scale=1.0)
                            # mask is non-unit only on the warm-up columns
                            nc.vector.tensor_mul(
                                xc[et][:, 0:W], xc[et][:, 0:W], mask_sb)

                    # issue: LN iterations with the first NW ets' in_proj
                    # chains interleaved per chunk, then the remaining ets
                    for it in range(NTT):
                        ln_iter(it)
                        if it % 3 == 2:
                            for e in range(NW):
                                s2_chain(e, it // 3)
                    for e in range(NW, KC):
                        for c in range(NTC):
                            s2_chain(e, c)

                    pstr_cm.__exit__(None, None, None)
                    psmm_cm.__exit__(None, None, None)

                    # psmm/pstr scopes closed: the z loop gets 6 PSUM banks so
                    # the PE isn't paced by the silu evacuations
                    with tc.tile_pool(name="psz", bufs=6, space="PSUM") as pszp:
                        # in_proj z-half (fp8 DoubleRow) + silu -> SBUF-resident
                        # zs (skip warm-up: the first chunk starts at token W);
                        # the silu activation scale de-quantizes x16 * x512
                        for et in range(KC):
                            wt = ws.tile([128, KD, 128], F8, tag="wz8", bufs=4,
                                         name="wtz")
                            nc.sync.dma_start(
                                out=wt,
                                in_=w1z_h.ap()[:, et * 128:(et + 1) * 128]
                                .rearrange("(kt p) e -> p kt e", p=128))
                            for tc3 in range(NTC):
                                lo = max(tc3 * TC, W)
                                n = (tc3 + 1) * TC - lo
                                off = lo - tc3 * TC
                                ps = pszp.tile([128, TC], F32, tag="mmz", name="psz")
                                for m in range(KD // 2):
                                    nc.tensor.matmul(
                                        ps[:, 0:n], wt[:, 2 * m:2 * m + 2, :],
                                        xT8[m][tc3][:, :, off:TC],
                                        start=(m == 0), stop=(m == KD // 2 - 1),
                                        perf_mode=DR)
                                nc.scalar.activation(
                                    out=zs[et][:, lo - W:(tc3 + 1) * TC - W],
                                    in_=ps[:, 0:n], func=AF.Silu,
                                    scale=1.0 / (16.0 * S_GW))

                        # force-live timing probes on idle engines: DVE bf16->fp8
                        # copy and gpsimd SBUF bf16 copy (rates decide future
                        # offloads); results go to a dram scratch nobody reads
                        prb8 = s1r.tile([128, NT], F8, tag="prb8", name="probe_dve8")
                        nc.vector.tensor_copy(prb8, xc[0])
                        nc.sync.dma_start(out=probe8_h.ap(), in_=prb8)
                        prbg = s1r.tile([128, NT], BF16, tag="prbg", name="probe_gp")
                        nc.gpsimd.tensor_copy(prbg, xc[1])
                        nc.sync.dma_start(out=probeg_h.ap(), in_=prbg)

                # ---- S4-S6: gate matmul (fp8 DoubleRow), sigmoid, chunked
                # scan, y*silu(z).  y is kept in SBUF as chunk tiles whose
                # column slices feed out_proj directly as stationary operands.
                with tc.tile_pool(name="yp", bufs=1) as yp, \
                     tc.tile_pool(name="opwp", bufs=1) as opp:
                    ych = [[None] * NTC for _ in range(KC)]
                    # full out_proj weight prefetch (bf16, 4.2MB) on the
                    # gpsimd queue while the PE runs the gate matmuls
                    opw_sb = opp.tile([128, KC, D], BF16, name="opw_sb")
                    nc.gpsimd.dma_start(
                        out=opw_sb,
                        in_=op_h.ap().rearrange("(kt p) d -> p kt d", p=128))

                    with tc.tile_pool(name="gws", bufs=3) as gs, \
                         tc.tile_pool(name="ach", bufs=6) as ayp, \
                         tc.tile_pool(name="s6roll", bufs=6) as s6r, \
                         tc.tile_pool(name="psg", bufs=4, space="PSUM") as psg:

                        for et in range(KC):
                            gt = gs.tile([128, KC, 128], F8, tag="gw", name="gt")
                            nc.sync.dma_start(
                                out=gt,
                                in_=gw_h.ap()[:, et * 128:(et + 1) * 128]
                                .rearrange("(kt p) e -> p kt e", p=128))
                            prev_y = None
                            for tc3 in range(NTC):
                                a_t = ayp.tile([128, TC], BF16, tag="ach", name="ach")
                                ps = psg.tile([128, TC], F32, tag="mm", name="psgt")
                                for j in range(KC // 2):
                                    nc.tensor.matmul(
                                        ps, gt[:, 2 * j:2 * j + 2, :],
                                        xc8[j][:, :, tc3 * TC:(tc3 + 1) * TC],
                                        start=(j == 0), stop=(j == KC // 2 - 1),
                                        perf_mode=DR)
                                nc.scalar.activation(
                                    out=a_t, in_=ps,
                                    func=AF.Sigmoid, bias=gateb[:, et:et + 1],
                                    scale=DEQ)
                                bt = s6r.tile([128, TC], BF16, tag="bt", name="bt")
                                nc.vector.scalar_tensor_tensor(
                                    out=bt, in0=a_t, scalar=1.0,
                                    in1=xc[et][:, tc3 * TC:(tc3 + 1) * TC],
                                    op0=OP.subtract, op1=OP.mult)
                                y_t = yp.tile([128, TC], BF16, name=f"y{et}_{tc3}")
                                init = 0.0 if tc3 == 0 else prev_y[:, TC - 1:TC]
                                nc.vector.tensor_tensor_scan(
                                    out=y_t, data0=a_t, data1=bt, initial=init,
                                    op0=OP.mult, op1=OP.add)
                                ych[et][tc3] = y_t
                                prev_y = y_t
                            # -yg: multiply after the carry chain is complete
                            for tc3 in range(NTC):
                                lo = max(tc3 * TC, W)
                                nc.vector.tensor_mul(
                                    ych[et][tc3][:, lo - tc3 * TC:TC],
                                    ych[et][tc3][:, lo - tc3 * TC:TC],
                                    zs[et][:, lo - W:(tc3 + 1) * TC - W])

                    # ---- S7: out_proj + residual.  y chunk slices are the
                    # stationary operands; 4 passes of (column half x 4 token
                    # blocks), 4 rolling PSUM banks so pass n+1 matmuls
                    # overlap pass n evac. ----
                    def yslice(kt, tb):
                        col = W + tb * 128          # absolute column in [0, NT)
                        tc3, off = col // TC, col % TC
                        return ych[kt][tc3][:, off:off + 128]

                    with tc.tile_pool(name="s7roll", bufs=6) as s7r, \
                         tc.tile_pool(name="s7res", bufs=8) as s7x, \
                         tc.tile_pool(name="psop", bufs=4, space="PSUM") as psop:
                        for nb in range(2):
                            for tbg in range(4):
                                tbs = [tbg * 2 + i for i in range(2)]
                                xres = {}
                                for tb in tbs:
                                    xres[tb] = s7x.tile([128, NB], F32, tag="xres",
                                                        name=f"xres{nb}_{tb}")
                                    nc.gpsimd.dma_start(
                                        out=xres[tb],
                                        in_=x_h.ap()[W + tb * 128:W + (tb + 1) * 128,
                                                     nb * NB:(nb + 1) * NB])
                                pss = {tb: psop.tile([128, NB], F32, tag="op",
                                                     name=f"pso{tb}") for tb in tbs}
                                for kt in range(KC):
                                    for tb in tbs:
                                        nc.tensor.matmul(
                                            pss[tb], yslice(kt, tb),
                                            opw_sb[:, kt, nb * NB:(nb + 1) * NB],
                                            start=(kt == 0), stop=(kt == KC - 1))
                                for tb in tbs:
                                    oh = s7r.tile([128, NB], F32, tag="oh", name="oh")
                                    nc.vector.tensor_sub(oh, xres[tb], pss[tb])
                                    nc.sync.dma_start(
                                        out=out_h.ap()[tb * 128:(tb + 1) * 128,
                                                       nb * NB:(nb + 1) * NB],
                                        in_=oh)

    nc.compile()
    return nc


def _prep_host(x, norm_w, norm_b, in_proj_w, conv_w, conv_b, gate_w, gate_b,
               out_proj_w):
    w1 = (in_proj_w * norm_w[None, :]).astype(np.float32)
    w1xT = np.ascontiguousarray(w1[:DI].T).astype(ml_dtypes.bfloat16)   # [D, DI]
    w1z8 = np.clip(np.ascontiguousarray(w1[DI:].T) * S_GW, -240,
                   240).astype(ml_dtypes.float8_e4m3)                   # [D, DI]
    gwT = np.ascontiguousarray(gate_w.T)             # [DI, DI]
    gw8 = np.clip(gwT * S_GW, -240, 240).astype(ml_dtypes.float8_e4m3)
    opT = np.ascontiguousarray(out_proj_w.T).astype(ml_dtypes.bfloat16)  # [DI, D]
    convw_r = np.ascontiguousarray(
        conv_w.reshape(KC, 128, 4).transpose(1, 0, 2).reshape(128, KC * 4)
    ).astype(ml_dtypes.bfloat16)
    convb_r = np.ascontiguousarray(conv_b.reshape(KC, 128).T)
    gateb_r = np.ascontiguousarray(gate_b.reshape(KC, 128).T)
    normb_r = np.ascontiguousarray(norm_b.reshape(KD, 128).T)

    in_maps = []
    for core in range(8):
        b, j = core // 4, core % 4
        xs = np.zeros((NT, D), np.float32)
        start = j * CHUNK - W
        mask = np.ones((1, NT), np.float32)
        if j == 0:
            xs[W:] = x[b, 0:CHUNK]
            mask[0, :W] = 0.0
        else:
            xs[:] = x[b, start:start + NT]
        in_maps.append({
            "x": np.ascontiguousarray(xs), "w1x": w1xT, "w1z8": w1z8,
            "gw": gw8, "opw": opT, "convw": convw_r, "convb": convb_r,
            "gateb": gateb_r, "normb": normb_r,
            "mask": mask.astype(ml_dtypes.bfloat16),
        })
    return in_maps


def kernel(x, norm_w, norm_b, in_proj_w, conv_w, conv_b, gate_w, gate_b,
           out_proj_w, _trace=False, _collect=None):
    x = np.asarray(x, np.float32)
    if "nc" not in _cache:
        _cache["nc"] = _build()
    nc = _cache["nc"]
    in_maps = _prep_host(
        x, np.asarray(norm_w, np.float32), np.asarray(norm_b, np.float32),
        np.asarray(in_proj_w, np.float32), np.asarray(conv_w, np.float32),
        np.asarray(conv_b, np.float32), np.asarray(gate_w, np.float32),
        np.asarray(gate_b, np.float32), np.asarray(out_proj_w, np.float32))
    res = run_bass_kernel_spmd(nc, in_maps, core_ids=list(range(8)), trace=_trace)
    if _collect is not None:
        _collect.append(res)
    out = np.empty((B, L, D), np.float32)
    for core in range(8):
        b, j = core // 4, core % 4
        out[b, j * CHUNK:(j + 1) * CHUNK] = res.results[core]["out"]
    return out


# revision 40
# speedup vs baseline: 1.6850x; 1.0007x over previous
"""GatedLinearRecurrence Trainium2 kernel (8-core SPMD, Bass/Tile).

Strategy: shard (batch=2) x (4 sequence chunks of 1024 tokens) across 8 cores.
Each core processes 1152 tokens: a 128-token warm-up window (re-computed
redundantly; the recurrence decay makes carry-in truncation error ~1e-24)
followed by its 1024 "main" tokens.  No collectives needed.

Per-core pipeline (channels-on-partitions, tokens-on-free layout):
  LN(x) [t,d] -> PE-transpose (bf16) -> x̂T [d,t] -> in_proj (bf16 matmul)
  -> causal depthwise conv (bf16, 4 shifted tensor_scalar ops) -> silu -> mask
  -> fp8 quantized copy -> gate matmul (fp8e4m3 DoubleRow, 2 k-tiles/instr)
  -> sigmoid (dequant via activation scale) -> b=-(1-a)*xc
  -> tensor_tensor_scan (h=-h, fp32 carry) -> y*silu(z) -> out_proj (bf16)
  -> residual subtract -> out [t,dm].

Dtype plan (validated numerically: rel err ~5e-3 vs 2e-2 budget):
  - gate matmul fp8e4m3 both operands (weights pre-scaled x512 on host,
    activations x16 in-kernel; sigmoid activation scale 1/8192 dequantizes).
    DoubleRow perf mode halves PE time for this largest matmul.
  - everything else bf16 (same PE rate as f32r, half the DMA + DVE cost).
  - silu(z) stays resident in SBUF (no HBM scratch roundtrip).

The sign trick: scan data1 = (a-1)*x_conv = -b gives -h; -h*silu(z) = -yg;
out = x - matmul(-yg) = x + proj(yg).
"""
import sys

for p in ("/opt/trn_rl_repo", "/root/.axon_site/_ro/trn_rl_repo"):
    if p not in sys.path:
        sys.path.insert(0, p)

import numpy as np
import ml_dtypes

import concourse.bass as bass
import concourse.bacc as bacc
import concourse.tile as tile
import concourse.mybir as mybir
from concourse.bass_utils import run_bass_kernel_spmd
from concourse.masks import make_identity

F32 = mybir.dt.float32
BF16 = mybir.dt.bfloat16
F8 = mybir.dt.float8e4
AF = mybir.ActivationFunctionType
OP = mybir.AluOpType
DR = mybir.MatmulPerfMode.DoubleRow

B, L, D = 2, 4096, 1024
DI = 2048            # d_inner
NT = 1152            # tokens per core (128 warm-up + 1024 main)
W = 128              # warm-up tokens
CHUNK = 1024
NTT = NT // 128      # 9 token tiles
KD = D // 128        # 8 k-tiles over d_model
KC = DI // 128       # 16 k-tiles over d_inner
TC = 384             # matmul N chunk (3 per core)
NTC = NT // TC
EPS = 1e-5
S_GW = 512.0         # host-side fp8 scale on gate weights
DEQ = 1.0 / S_GW     # x_conv is quantized to fp8 unscaled (values < 10)
NB = 512             # out_proj column half width

_cache = {}


def _build():
    nc = bacc.Bacc(None, target_bir_lowering=False)

    x_h = nc.dram_tensor("x", [NT, D], F32, kind="ExternalInput")
    w1x_h = nc.dram_tensor("w1x", [D, DI], BF16, kind="ExternalInput")
    w1z_h = nc.dram_tensor("w1z8", [D, DI], F8, kind="ExternalInput")
    gw_h = nc.dram_tensor("gw", [DI, DI], F8, kind="ExternalInput")
    op_h = nc.dram_tensor("opw", [DI, D], BF16, kind="ExternalInput")
    convw_h = nc.dram_tensor("convw", [128, KC * 4], BF16, kind="ExternalInput")
    convb_h = nc.dram_tensor("convb", [128, KC], F32, kind="ExternalInput")
    gateb_h = nc.dram_tensor("gateb", [128, KC], F32, kind="ExternalInput")
    normb_h = nc.dram_tensor("normb", [128, KD], F32, kind="ExternalInput")
    mask_h = nc.dram_tensor("mask", [1, NT], BF16, kind="ExternalInput")
    out_h = nc.dram_tensor("out", [CHUNK, D], F32, kind="ExternalOutput")
    probe8_h = nc.dram_tensor("probe8", [128, NT], F8, kind="Internal")
    probeg_h = nc.dram_tensor("probeg", [128, NT], BF16, kind="Internal")

    with tile.TileContext(nc) as tc:
        with tc.tile_pool(name="consts", bufs=1) as consts:

            ident = consts.tile([128, 128], BF16, name="ident")
            make_identity(nc, ident)
            mask_sb = consts.tile([128, W], BF16, name="mask_sb")
            nc.gpsimd.dma_start(
                out=mask_sb,
                in_=bass.AP(tensor=mask_h, offset=0, ap=[[0, 128], [1, W]]),
            )
            convw = consts.tile([128, KC * 4], BF16, name="convw")
            nc.gpsimd.dma_start(out=convw, in_=convw_h.ap())
            convb = consts.tile([128, KC], F32, name="convb")
            nc.gpsimd.dma_start(out=convb, in_=convb_h.ap())
            gateb = consts.tile([128, KC], F32, name="gateb")
            nc.gpsimd.dma_start(out=gateb, in_=gateb_h.ap())
            normb = consts.tile([128, KD], F32, name="normb")
            nc.gpsimd.dma_start(out=normb, in_=normb_h.ap())
            eps_t = consts.tile([128, 1], F32, name="eps_t")
            nc.vector.memset(eps_t, EPS)
            # pre-warm the activation function tables so the first LN tile
            # doesn't eat a 1.3us ACT_TABLE_LOAD on the critical path
            warm = consts.tile([128, 4], F32, name="warm")
            nc.scalar.activation(out=warm[:, 0:1], in_=eps_t, func=AF.Sqrt)
            nc.scalar.activation(out=warm[:, 1:2], in_=eps_t, func=AF.Identity)
            nc.scalar.activation(out=warm[:, 2:3], in_=eps_t, func=AF.Silu)
            nc.scalar.activation(out=warm[:, 3:4], in_=eps_t, func=AF.Sigmoid)
            zeros_bf = consts.tile([128, NT], BF16, name="zeros_bf")
            nc.vector.memset(zeros_bf, 0.0)

            with tc.tile_pool(name="xcp", bufs=1) as xcp, \
                 tc.tile_pool(name="zsp", bufs=1) as zsp, \
                 tc.tile_pool(name="opwp", bufs=1) as opp:
                xc = [xcp.tile([128, NT], BF16, name=f"xct{e}") for e in range(KC)]
                # fp8 pair tiles for the DoubleRow gate matmul: pair j holds
                # channel tiles 2j (dim1=0) and 2j+1 (dim1=1), scaled by S_XC
                xc8 = [xcp.tile([128, 2, NT], F8, name=f"xc8_{j}")
                       for j in range(KC // 2)]
                zs = [zsp.tile([128, CHUNK], BF16, name=f"zs{e}") for e in range(KC)]

                # ---- S1-S3: LN, transpose, in_proj (x & z), conv, silu ----
                NW = 6
                with tc.tile_pool(name="xT", bufs=1) as xTp, \
                     tc.tile_pool(name="s1roll", bufs=2) as s1r, \
                     tc.tile_pool(name="stat", bufs=4) as stp, \
                     tc.tile_pool(name="w1s", bufs=3) as ws:

                    # x-hat-T chunk tiles [d-tile][t-chunk]: finer deps, so
                    # the first in_proj matmuls start after 3 LN iterations.
                    xT = [[xTp.tile([128, TC], BF16, name=f"xTt{d_}_{c_}")
                           for c_ in range(NTC)] for d_ in range(KD)]
                    # fp8 x-hat pair tiles for the DoubleRow z matmul: pair dp
                    # holds d-tiles 2dp/2dp+1, scaled x16
                    xT8 = [[xTp.tile([128, 2, TC], F8, name=f"xT8_{dp}_{c_}")
                            for c_ in range(NTC)] for dp in range(KD // 2)]

                    # prefetch the first in_proj weight tiles on the (idle)
                    # gpsimd queue: DMA trigger instructions cost ~2us of
                    # engine time on the scalar queue, which would sit in
                    # front of the first layernorm Sqrt
                    psmm_cm = tc.tile_pool(name="psmm", bufs=4, space="PSUM")
                    pstr_cm = tc.tile_pool(name="pstr", bufs=2, space="PSUM")
                    psmm = psmm_cm.__enter__()
                    pstr = pstr_cm.__enter__()

                    wts, xins = {}, {}
                    for et in range(NW):
                        wt = ws.tile([128, KD, 128], BF16, tag="w1",
                                     bufs=8, name=f"wt{et}")
                        nc.gpsimd.dma_start(
                            out=wt,
                            in_=w1x_h.ap()[:, et * 128:(et + 1) * 128]
                            .rearrange("(kt p) e -> p kt e", p=128))
                        wts[et] = wt

                    def ln_iter(it):
                        tc3, col = it // 3, (it % 3) * 128
                        xt = s1r.tile([128, D], F32, tag="xt", bufs=2, name="xt")
                        if it == 0:
                            # split so the first bn_stats starts a half earlier
                            nc.sync.dma_start(out=xt[:, 0:512],
                                              in_=x_h.ap()[0:128, 0:512])
                            nc.sync.dma_start(out=xt[:, 512:1024],
                                              in_=x_h.ap()[0:128, 512:1024])
                        else:
                            nc.sync.dma_start(
                                out=xt, in_=x_h.ap()[it * 128:(it + 1) * 128, :])
                        stats = stp.tile([128, 2, 6], F32, tag="stats", name="stats")
                        nc.vector.bn_stats(out=stats[:, 0, :], in_=xt[:, 0:512])
                        nc.vector.bn_stats(out=stats[:, 1, :], in_=xt[:, 512:1024])
                        mv = stp.tile([128, 2], F32, tag="mv", name="mv")
                        nc.vector.bn_aggr(out=mv, in_=stats)
                        rstd = stp.tile([128, 1], F32, tag="rstd", name="rstd")
                        nc.scalar.activation(out=rstd, in_=mv[:, 1:2], func=AF.Sqrt,
                                             bias=eps_t, scale=1.0)
                        nc.vector.reciprocal(out=rstd, in_=rstd)
                        xtn = s1r.tile([128, D], BF16, tag="xtn", bufs=3, name="xtn")
                        nc.vector.tensor_scalar(out=xtn, in0=xt, scalar1=mv[:, 0:1],
                                                scalar2=rstd, op0=OP.subtract, op1=OP.mult)
                        for d_ in range(KD):
                            pst = pstr.tile([128, 128], BF16, tag="tr", name="pst")
                            nc.tensor.transpose(pst, xtn[:, d_ * 128:(d_ + 1) * 128], ident)
                            # evac + norm_b on the Scalar engine
                            nc.scalar.activation(
                                out=xT[d_][tc3][:, col:col + 128], in_=pst,
                                func=AF.Identity, bias=normb[:, d_:d_ + 1], scale=1.0)


                    # in_proj x-half + conv + silu + warm-up mask + fp8 copy.
                    # The first NW ets interleave their t-chunks (issued
                    # inside the LN loop) so the PE never waits on layernorm
                    # chunks still in flight.
                    def s2_chain(et, tc3):
                        if tc3 == 0:
                            if et not in wts:
                                wt = ws.tile([128, KD, 128], BF16, tag="w1",
                                             bufs=8, name=f"wt{et}")
                                nc.sync.dma_start(
                                    out=wt,
                                    in_=w1x_h.ap()[:, et * 128:(et + 1) * 128]
                                    .rearrange("(kt p) e -> p kt e", p=128))
                                wts[et] = wt
                            xin = s1r.tile([128, NT + 3], BF16, tag="xin",
                                           bufs=8, name=f"xin{et}")
                            nc.vector.memset(xin[:, 0:3], 0.0)
                            xins[et] = xin
                        ps = psmm.tile([128, TC], F32, tag="mm", name="ps")
                        for kt in range(KD):
                            nc.tensor.matmul(
                                ps, wts[et][:, kt, :], xT[kt][tc3],
                                start=(kt == 0), stop=(kt == KD - 1))
                        nc.scalar.copy(
                            out=xins[et][:, 3 + tc3 * TC: 3 + (tc3 + 1) * TC], in_=ps)
                        if tc3 == NTC - 1:
                            wts.pop(et)
                            xin = xins.pop(et)
                            tmp = s1r.tile([128, NT], BF16, tag="ctmp", name="ctmp")
                            for k in range(4):
                                nc.vector.scalar_tensor_tensor(
                                    out=tmp, in0=xin[:, k:k + NT],
                                    scalar=convw[:, et * 4 + k:et * 4 + k + 1],
                                    in1=zeros_bf if k == 0 else tmp,
                                    op0=OP.mult, op1=OP.add)
                            nc.scalar.activation(out=xc[et], in_=tmp, func=AF.Silu,
                                                 bias=convb[:, et:et + 1], scale=1.0)
                            # mask is non-unit only on the warm-up columns
                            nc.vector.tensor_mul(
                                xc[et][:, 0:W], xc[et][:, 0:W], mask_sb)

                    # issue: LN iterations with the first NW ets' in_proj
                    # chains interleaved per chunk, then the remaining ets
                    for it in range(NTT):
                        ln_iter(it)
                        if it % 3 == 2:
                            for e in range(NW):
                                s2_chain(e, it // 3)
                    for e in range(NW, KC):
                        for c in range(NTC):
                            s2_chain(e, c)

                    pstr_cm.__exit__(None, None, None)
                    psmm_cm.__exit__(None, None, None)

                    # psmm/pstr scopes closed: the z loop gets 6 PSUM banks so
                    # the PE isn't paced by the silu evacuations
                    with tc.tile_pool(name="psz", bufs=6, space="PSUM") as pszp:
                        # deferred fp8 emissions, issued BEHIND the x-half
                        # copies/silus on the scalar queue so they never delay
                        # the conv chain: x-hat pair tiles for z (scale 16)
                        # and gate operand copies from the masked xc
                        for tc3 in range(NTC):
                            for dp in range(KD // 2):
                                for i in range(2):
                                    nc.scalar.activation(
                                        out=xT8[dp][tc3][:, i, :],
                                        in_=xT[2 * dp + i][tc3], func=AF.Identity,
                                        scale=16.0)
                        for et in range(KC):
                            nc.scalar.activation(
                                out=xc8[et // 2][:, et % 2, :], in_=xc[et],
                                func=AF.Identity, scale=1.0)

                        # in_proj z-half (fp8 DoubleRow) + silu -> SBUF-resident
                        # zs (skip warm-up: the first chunk starts at token W);
                        # the silu activation scale de-quantizes x16 * x512
                        for et in range(KC):
                            wt = ws.tile([128, KD, 128], F8, tag="wz8", bufs=4,
                                         name="wtz")
                            nc.sync.dma_start(
                                out=wt,
                                in_=w1z_h.ap()[:, et * 128:(et + 1) * 128]
                                .rearrange("(kt p) e -> p kt e", p=128))
                            for tc3 in range(NTC):
                                lo = max(tc3 * TC, W)
                                n = (tc3 + 1) * TC - lo
                                off = lo - tc3 * TC
                                ps = pszp.tile([128, TC], F32, tag="mmz", name="psz")
                                for m in range(KD // 2):
                                    nc.tensor.matmul(
                                        ps[:, 0:n], wt[:, 2 * m:2 * m + 2, :],
                                        xT8[m][tc3][:, :, off:TC],
                                        start=(m == 0), stop=(m == KD // 2 - 1),
                                        perf_mode=DR)
                                nc.scalar.activation(
                                    out=zs[et][:, lo - W:(tc3 + 1) * TC - W],
                                    in_=ps[:, 0:n], func=AF.Silu,
                                    scale=1.0 / (16.0 * S_GW))

                        # full out_proj weight prefetch (bf16, 4.2MB): issued
                        # here so the transfer streams during the z phase
                        # instead of colliding with the gate-weight loads
                        opw_sb = opp.tile([128, KC, D], BF16, name="opw_sb")
                        nc.gpsimd.dma_start(
                            out=opw_sb,
                            in_=op_h.ap().rearrange("(kt p) d -> p kt d", p=128))

                # ---- S4-S6: gate matmul (fp8 DoubleRow), sigmoid, chunked
                # scan, y*silu(z).  y is kept in SBUF as chunk tiles whose
                # column slices feed out_proj directly as stationary operands.
                with tc.tile_pool(name="yp", bufs=1) as yp:
                    ych = [[None] * NTC for _ in range(KC)]

                    with tc.tile_pool(name="gws", bufs=4) as gs, \
                         tc.tile_pool(name="ach", bufs=6) as ayp, \
                         tc.tile_pool(name="s6roll", bufs=6) as s6r, \
                         tc.tile_pool(name="psg", bufs=4, space="PSUM") as psg:

                        for et in range(KC):
                            gt = gs.tile([128, KC, 128], F8, tag="gw", name="gt")
                            nc.sync.dma_start(
                                out=gt,
                                in_=gw_h.ap()[:, et * 128:(et + 1) * 128]
                                .rearrange("(kt p) e -> p kt e", p=128))
                            prev_y = None
                            for tc3 in range(NTC):
                                a_t = ayp.tile([128, TC], BF16, tag="ach", name="ach")
                                ps = psg.tile([128, TC], F32, tag="mm", name="psgt")
                                for j in range(KC // 2):
                                    nc.tensor.matmul(
                                        ps, gt[:, 2 * j:2 * j + 2, :],
                                        xc8[j][:, :, tc3 * TC:(tc3 + 1) * TC],
                                        start=(j == 0), stop=(j == KC // 2 - 1),
                                        perf_mode=DR)
                                nc.scalar.activation(
                                    out=a_t, in_=ps,
                                    func=AF.Sigmoid, bias=gateb[:, et:et + 1],
                                    scale=DEQ)
                                bt = s6r.tile([128, TC], BF16, tag="bt", name="bt")
                                nc.vector.scalar_tensor_tensor(
                                    out=bt, in0=a_t, scalar=1.0,
                                    in1=xc[et][:, tc3 * TC:(tc3 + 1) * TC],
                                    op0=OP.subtract, op1=OP.mult)
                                y_t = yp.tile([128, TC], BF16, name=f"y{et}_{tc3}")
                                init = 0.0 if tc3 == 0 else prev_y[:, TC - 1:TC]
                                nc.vector.tensor_tensor_scan(
                                    out=y_t, data0=a_t, data1=bt, initial=init,
                                    op0=OP.mult, op1=OP.add)
                                ych[et][tc3] = y_t
                                prev_y = y_t
                            # -yg: multiply after the carry chain is complete
                            for tc3 in range(NTC):
                                lo = max(tc3 * TC, W)
                                nc.vector.tensor_mul(
                                    ych[et][tc3][:, lo - tc3 * TC:TC],
                                    ych[et][tc3][:, lo - tc3 * TC:TC],
                                    zs[et][:, lo - W:(tc3 + 1) * TC - W])

                    # ---- S7: out_proj + residual.  y chunk slices are the
                    # stationary operands; 4 passes of (column half x 4 token
                    # blocks), 4 rolling PSUM banks so pass n+1 matmuls
                    # overlap pass n evac. ----
                    def yslice(kt, tb):
                        col = W + tb * 128          # absolute column in [0, NT)
                        tc3, off = col // TC, col % TC
                        return ych[kt][tc3][:, off:off + 128]

                    with tc.tile_pool(name="s7roll", bufs=6) as s7r, \
                         tc.tile_pool(name="s7res", bufs=8) as s7x, \
                         tc.tile_pool(name="psop", bufs=4, space="PSUM") as psop:
                        for nb in range(2):
                            for tbg in range(4):
                                tbs = [tbg * 2 + i for i in range(2)]
                                xres = {}
                                for tb in tbs:
                                    xres[tb] = s7x.tile([128, NB], F32, tag="xres",
                                                        name=f"xres{nb}_{tb}")
                                    nc.gpsimd.dma_start(
                                        out=xres[tb],
                                        in_=x_h.ap()[W + tb * 128:W + (tb + 1) * 128,
                                                     nb * NB:(nb + 1) * NB])
                                pss = {tb: psop.tile([128, NB], F32, tag="op",
                                                     name=f"pso{tb}") for tb in tbs}
                                for kt in range(KC):
                                    for tb in tbs:
                                        nc.tensor.matmul(
                                            pss[tb], yslice(kt, tb),
                                            opw_sb[:, kt, nb * NB:(nb + 1) * NB],
                                            start=(kt == 0), stop=(kt == KC - 1))
                                for tb in tbs:
                                    oh = s7r.tile([128, NB], F32, tag="oh", name="oh")
                                    nc.vector.tensor_sub(oh, xres[tb], pss[tb])
                                    nc.sync.dma_start(
                                        out=out_h.ap()[tb * 128:(tb + 1) * 128,
                                                       nb * NB:(nb + 1) * NB],
                                        in_=oh)

    nc.compile()
    return nc


def _prep_host(x, norm_w, norm_b, in_proj_w, conv_w, conv_b, gate_w, gate_b,
               out_proj_w):
    w1 = (in_proj_w * norm_w[None, :]).astype(np.float32)
    w1xT = np.ascontiguousarray(w1[:DI].T).astype(ml_dtypes.bfloat16)   # [D, DI]
    w1z8 = np.clip(np.ascontiguousarray(w1[DI:].T) * S_GW, -240,
                   240).astype(ml_dtypes.float8_e4m3)                   # [D, DI]
    gwT = np.ascontiguousarray(gate_w.T)             # [DI, DI]
    gw8 = np.clip(gwT * S_GW, -240, 240).astype(ml_dtypes.float8_e4m3)
    opT = np.ascontiguousarray(out_proj_w.T).astype(ml_dtypes.bfloat16)  # [DI, D]
    convw_r = np.ascontiguousarray(
        conv_w.reshape(KC, 128, 4).transpose(1, 0, 2).reshape(128, KC * 4)
    ).astype(ml_dtypes.bfloat16)
    convb_r = np.ascontiguousarray(conv_b.reshape(KC, 128).T)
    gateb_r = np.ascontiguousarray(gate_b.reshape(KC, 128).T)
    normb_r = np.ascontiguousarray(norm_b.reshape(KD, 128).T)

    in_maps = []
    for core in range(8):
        b, j = core // 4, core % 4
        xs = np.zeros((NT, D), np.float32)
        start = j * CHUNK - W
        mask = np.ones((1, NT), np.float32)
        if j == 0:
            xs[W:] = x[b, 0:CHUNK]
            mask[0, :W] = 0.0
        else:
            xs[:] = x[b, start:start + NT]
        in_maps.append({
            "x": np.ascontiguousarray(xs), "w1x": w1xT, "w1z8": w1z8,
            "gw": gw8, "opw": opT, "convw": convw_r, "convb": convb_r,
            "gateb": gateb_r, "normb": normb_r,
            "mask": mask.astype(ml_dtypes.bfloat16),
        })
    return in_maps


def kernel(x, norm_w, norm_b, in_proj_w, conv_w, conv_b, gate_w, gate_b,
           out_proj_w, _trace=False, _collect=None):
    x = np.asarray(x, np.float32)
    if "nc" not in _cache:
        _cache["nc"] = _build()
    nc = _cache["nc"]
    in_maps = _prep_host(
        x, np.asarray(norm_w, np.float32), np.asarray(norm_b, np.float32),
        np.asarray(in_proj_w, np.float32), np.asarray(conv_w, np.float32),
        np.asarray(conv_b, np.float32), np.asarray(gate_w, np.float32),
        np.asarray(gate_b, np.float32), np.asarray(out_proj_w, np.float32))
    res = run_bass_kernel_spmd(nc, in_maps, core_ids=list(range(8)), trace=_trace)
    if _collect is not None:
        _collect.append(res)
    out = np.empty((B, L, D), np.float32)
    for core in range(8):
        b, j = core // 4, core % 4
        out[b, j * CHUNK:(j + 1) * CHUNK] = res.results[core]["out"]
    return out
